# revision 1
# baseline (speedup 1.0000x reference)
"""Trainium2 Bass/Tile kernel for AttnBlock:
GroupNorm(32) -> 1x1 conv q,k,v -> full softmax attention over N=4096 tokens
-> 1x1 conv proj -> residual.

Sharding: 8 cores = 2 (batch) x 4 (query-token shards of N).  Each core gets
the full [C, N] image of its batch (to compute GroupNorm stats and full k/v)
plus its n-shard slice, and produces the [C, N/4] output shard.  No
collectives.

GroupNorm is folded into the qkv weights on-device:
    h = a*x + b  (a = rstd*gamma per channel, b = beta - mu*a)
    q = w0 @ h + b0 = (w0*a) @ x + (w0 @ b + b0)
Attention uses the transposed-score layout S_T[m, n] = sum_c k[c,m] q[c,n]
so softmax-exp output E feeds the AV matmul as the stationary operand with
no transposes; the softmax denominator comes free from an appended
ones-column on v^T.  Softmax max-subtraction is skipped (logits here are
|logit| < 10 by construction; exp is fp32-safe).

Big matmuls run in float32r (4x the fp32 rate at free-dim >= 256).  Tensors
feeding those matmuls are declared float32r so their producers (DMA / ACT /
DVE) satisfy the BIR "rounded to FP32r" rule; fp32 views are obtained via
bitcast where full-precision elementwise/matmul semantics are needed.
"""

import ml_dtypes
import numpy as np

import concourse.bacc as bacc
import concourse.bass as bass
import concourse.mybir as mybir
import concourse.tile as tile
from concourse import bass_utils

f32 = mybir.dt.float32
f32r = mybir.dt.float32r
bf16 = mybir.dt.bfloat16
AF = mybir.ActivationFunctionType
ALU = mybir.AluOpType
AX = mybir.AxisListType

B = 2
C = 256
N = 4096          # 16**3 tokens
NSH = N // 4      # 1024 tokens per core
G = 32
CPG = C // G      # channels per group
NPG = CPG * N     # elements per group
EPS = 1e-6
SCALE = C ** -0.5
NCORES = 8

USE_F32R = True

DT = f32r if USE_F32R else f32

# hardware-bisection aid (edit by hand when debugging): 0/8/9/11/12/13 stop
# the kernel early at successive stages; 4 = full kernel.
PHASE = 4


def _F(ap):
    """fp32 view of a (possibly f32r) AP, for elementwise/fp32-matmul use."""
    return ap.bitcast(f32) if USE_F32R else ap


def _build_body(nc, tc, d):
    """Emit the kernel body. d: dict of dram APs."""
    from contextlib import ExitStack

    ctx = ExitStack()
    pc = ctx.enter_context(tc.tile_pool(name="const", bufs=1))
    pb = ctx.enter_context(tc.tile_pool(name="big", bufs=1))
    pw = ctx.enter_context(tc.tile_pool(name="work", bufs=3))
    ptiny = ctx.enter_context(tc.tile_pool(name="tiny", bufs=2))
    # PSUM budget is 8 banks of [128, 512] f32; every tile below is <= 1 bank.
    # Static reservation: mm512(2) + misc(2) + ot(4) = 8 banks.
    ps512 = ctx.enter_context(tc.tile_pool(name="ps512", bufs=2, space="PSUM"))
    ps256 = ctx.enter_context(tc.tile_pool(name="ps256", bufs=2, space="PSUM"))
    pot = ctx.enter_context(tc.tile_pool(name="pot", bufs=4, space="PSUM"))

    # ---- constants ----
    # zero column registered as the const-AP bias used implicitly by
    # non-Copy activations (Exp, Identity-with-accum, ...)
    zcol = pc.tile([128, 1], f32, tag="zcol", name="zcol")
    nc.vector.memset(zcol[:], 0.0)
    nc.const_aps.aps[(f32, 0.0)] = zcol[:]
    epscol = pc.tile([16, 1], f32, tag="epscol", name="epscol")
    nc.vector.memset(epscol[:], EPS)
    ones2 = pc.tile([128, 2], f32, tag="ones2", name="ones2")
    nc.vector.memset(ones2[:], 1.0)

    # packed constants: cols [cvec0(8) | cvec1(8) | gmask(16) | gmaskT(128,
    # rows 0..15) | ident(128)] -> one fp32 DMA
    cpack = pc.tile([128, 288], f32, tag="cpack", name="cpack")
    nc.scalar.dma_start(cpack[:], d["cpack"][:])
    cvec = [cpack[:, t * 8:(t + 1) * 8] for t in range(2)]
    gmask = cpack[:, 16:32]
    gmaskT = cpack[0:16, 32:160]
    ident = cpack[:, 160:288]

    # ---- PE warmup: ~40 independent matmuls on the identity tile bridge the
    # DMA/stats head so the HAM clock gate never throttles the real work.
    for i in range(40):
        wp = ps512.tile([128, 128], f32, tag="mm512", name="mm512")
        nc.tensor.matmul(wp[:], ident, ident, start=True, stop=True)

    # ---- x (full batch image) first: GroupNorm stats are the critical path.
    # One whole-tile DMA each (per-dma sequencer overhead dominates chunking).
    xs = [pb.tile([128, N], bf16, tag=f"xs{t}", name=f"xs{t}") for t in range(2)]
    NCH = 8  # 512-wide stat chunks
    for t in range(2):
        nc.sync.dma_start(xs[t][:], d["x"][t * 128:(t + 1) * 128, :])
    xq = [pb.tile([128, NSH], f32, tag=f"xq{t}", name=f"xq{t}") for t in range(2)]
    for t in range(2):
        nc.scalar.dma_start(xq[t][:], d["xq"][t * 128:(t + 1) * 128, :])
    # rounded copy for the bf16 q-matmul (GpSimd: off the ACT/DVE hot paths)
    xqr = [pb.tile([128, NSH], bf16, tag=f"xqr{t}", name=f"xqr{t}") for t in range(2)]
    for t in range(2):
        nc.gpsimd.tensor_copy(xqr[t][:], xq[t][:])

    # ---- weights (pre-transposed on host: w{i}t[ci, o] = w{i}[o, ci]);
    # w0..w2 packed into one bf16 tensor, w3 separate (f32r) ----
    wb = [pb.tile([128, 3 * C], bf16, tag=f"wb{t}", name=f"wb{t}")
          for t in range(2)]
    for t in range(2):
        nc.scalar.dma_start(wb[t][:], d["wbt"][t * 128:(t + 1) * 128, :])
    w3 = [pb.tile([128, C], DT, tag=f"w3_{t}", name=f"w3_{t}")
          for t in range(2)]
    for t in range(2):
        nc.scalar.dma_start(w3[t][:], d["w3t"][t * 128:(t + 1) * 128, :])
    w_sb = [[wb[t][:, i * C:(i + 1) * C] for t in range(2)] for i in range(3)]
    w_sb.append([w3[t][:] for t in range(2)])

    def _early_out0(srcs):
        """DMA a [2]x[128, NSH] pair straight to y (phase bisection)."""
        for oh in range(2):
            for ch in range(2):
                yt = pw.tile([128, 512], f32, tag="yt", name="yt")
                nc.vector.tensor_copy(yt[:], _F(srcs[oh][:, ch * 512:(ch + 1) * 512]))
                nc.sync.dma_start(
                    d["y"][oh * 128:(oh + 1) * 128, ch * 512:(ch + 1) * 512],
                    yt[:],
                )

    if PHASE == 0:
        _early_out0(xq)
        ctx.close()
        return

    # ---- GroupNorm statistics ----
    # per-partition partial sums: cols 0..7 = sum(x) per 512-chunk,
    # cols 8..15 = sum(x^2) per 512-chunk
    pt = [pb.tile([128, 16], f32, tag=f"pt{t}", name=f"pt{t}") for t in range(2)]
    for t in range(2):
        for ch in range(NCH):
            chunk = xs[t][:, ch * 512:(ch + 1) * 512]
            # NOTE: DVE tensor_tensor_reduce wedges the device (verified by
            # bisection).  ACT does all sum-of-squares via Square+accum_out
            # (one pass), DVE does the plain sums: balanced ~7us each.
            trashV = pw.tile([128, 512], f32, tag="trashV", name="trashV",
                             bufs=2)
            nc.scalar.activation(
                trashV[:], chunk, AF.Square,
                accum_out=pt[t][:, 8 + ch:9 + ch],
            )
            nc.vector.reduce_sum(pt[t][:, ch:ch + 1], chunk, axis=AX.X)

    if PHASE in (8, 9, 11):
        _early_out0(xq)
        ctx.close()
        return

    # group-combine: stats_ps[g, t*16 + (0..7 sum | 8..15 sumsq)]
    stats_ps = ps256.tile([16, 32], f32, tag="p256", name="p256")
    for t in range(2):
        nc.tensor.matmul(
            stats_ps[:, t * 16:(t + 1) * 16], gmask, pt[t][:],
            start=True, stop=True,
        )

    # per-group mean / rstd, both c-tiles vectorized.
    # mr4 = [mu_t0 | rstd_t0 | mu_t1 | rstd_t1]  [16, 4]
    mr4 = ptiny.tile([16, 4], f32, tag="mr4", name="mr4")
    mr4v = mr4[:].rearrange("p (t k) -> p t k", k=2)
    s16 = stats_ps[:].rearrange("p (t x) -> p t x", x=16)
    ex2 = ptiny.tile([16, 2], f32, tag="ex2", name="ex2")
    ex2v = ex2[:].rearrange("p (t k) -> p t k", k=1)
    nc.vector.reduce_sum(mr4v[:, :, 0:1], s16[:, :, 0:8], axis=AX.X)
    nc.vector.reduce_sum(ex2v[:], s16[:, :, 8:16], axis=AX.X)
    musq = ptiny.tile([16, 2], f32, tag="musq", name="musq")
    musqv = musq[:].rearrange("p (t k) -> p t k", k=1)
    nc.vector.tensor_mul(musqv[:], mr4v[:, :, 0:1], mr4v[:, :, 0:1])
    var = ptiny.tile([16, 2], f32, tag="var", name="var")
    varv = var[:].rearrange("p (t k) -> p t k", k=1)
    nc.vector.tensor_sub(varv[:], ex2v[:], musqv[:])
    std = ptiny.tile([16, 2], f32, tag="std", name="std")
    stdv = std[:].rearrange("p (t k) -> p t k", k=1)
    nc.scalar.activation(stdv[:], varv[:], AF.Sqrt, bias=epscol[:])
    nc.vector.reciprocal(mr4v[:, :, 1:2], stdv[:])
    # overwrite the mu column with mu*rstd (needed for the bfold fold below)
    nc.vector.tensor_mul(mr4v[:, :, 0:1], mr4v[:, :, 0:1], mr4v[:, :, 1:2])

    if PHASE == 12:
        _early_out0(xq)
        ctx.close()
        return

    # broadcast back to channels in one matmul:
    # bc[:, 2t+0/1] = per-channel [mu | rstd] for c-tile t
    a_t = []      # rstd*gamma per channel
    bfold = []    # beta - mu*rstd*gamma per channel
    bc_ps = ps256.tile([128, 4], f32, tag="p256", name="p256")
    nc.tensor.matmul(bc_ps[:], gmaskT, mr4[:], start=True, stop=True)
    for t in range(2):
        a = pb.tile([128, 1], f32, tag=f"a{t}", name=f"a{t}")
        nc.vector.tensor_mul(a[:], bc_ps[:, 2 * t + 1:2 * t + 2], cvec[t][:, 0:1])
        bf = pb.tile([128, 1], f32, tag=f"bf{t}", name=f"bf{t}")
        nc.vector.tensor_scalar(
            bf[:], bc_ps[:, 2 * t:2 * t + 1], cvec[t][:, 6:7], cvec[t][:, 1:2],
            op0=ALU.mult, op1=ALU.add,
        )
        a_t.append(a)
        bfold.append(bf)

    if PHASE == 13:
        _early_out0(xq)
        ctx.close()
        return

    # ---- effective biases (use UNSCALED weights; emitted before scaling) ----
    # beff_i[o] = sum_ci w_i[o,ci]*bfold[ci] + b_i[o]   for i in 0,1,2
    beff = []  # beff[i][oh] : [128, 1]
    bfold_bf = []
    for t in range(2):
        bb = pb.tile([128, 1], bf16, tag=f"bfb{t}", name=f"bfb{t}")
        nc.vector.tensor_copy(bb[:], bfold[t][:])
        bfold_bf.append(bb)
    for i in range(3):
        per_oh = []
        for oh in range(2):
            bp = ps256.tile([128, 1], f32, tag="p256", name="p256")
            for t in range(2):
                nc.tensor.matmul(
                    bp[:], w_sb[i][t][:, oh * 128:(oh + 1) * 128], bfold_bf[t][:],
                    start=(t == 0), stop=(t == 1),
                )
            bs = pb.tile([128, 1], f32, tag=f"beff{i}_{oh}", name=f"beff{i}_{oh}")
            nc.scalar.activation(bs[:], bp[:], AF.Identity,
                                 bias=cvec[oh][:, 2 + i:3 + i])
            per_oh.append(bs)
        beff.append(per_oh)
    # b3eff[o] = sum_c w3[o,c]*b2eff[c] + b3[o]
    b3eff = []
    for oh in range(2):
        bp = ps256.tile([128, 1], f32, tag="p256", name="p256")
        for t in range(2):
            nc.tensor.matmul(
                bp[:], _F(w_sb[3][t][:, oh * 128:(oh + 1) * 128]), beff[2][t][:],
                start=(t == 0), stop=(t == 1),
            )
        bs = pb.tile([128, 1], f32, tag=f"b3eff{oh}", name=f"b3eff{oh}")
        nc.scalar.activation(bs[:], bp[:], AF.Identity,
                             bias=cvec[oh][:, 5:6])
        b3eff.append(bs)

    # ---- fold GroupNorm scale into qkv weights (in place; DVE writes f32r,
    # satisfying the rounded-to-FP32r rule) ----
    for i in range(3):
        for t in range(2):
            nc.vector.tensor_scalar_mul(w_sb[i][t], w_sb[i][t], a_t[t][:])

    def _early_out(srcs):
        """DMA a [2]x[128, NSH] pair straight to y (phase bisection)."""
        for oh in range(2):
            for ch in range(2):
                yt = pw.tile([128, 512], f32, tag="yt", name="yt")
                nc.vector.tensor_copy(yt[:], _F(srcs[oh][:, ch * 512:(ch + 1) * 512]))
                nc.sync.dma_start(
                    d["y"][oh * 128:(oh + 1) * 128, ch * 512:(ch + 1) * 512],
                    yt[:],
                )

    if PHASE == 1:
        # y = xq * a + bfold  (the folded GroupNorm, applied directly)
        hq = [pb.tile([128, NSH], f32, tag=f"hq{t}", name=f"hq{t}")
              for t in range(2)]
        for t in range(2):
            nc.vector.tensor_scalar(
                hq[t][:], _F(xq[t][:]), a_t[t][:], bfold[t][:],
                op0=ALU.mult, op1=ALU.add,
            )
        _early_out(hq)
        ctx.close()
        return

    # ---- q = w0' @ xq + beff0 : [C(2 tiles), NSH] ----
    q_sb = [pb.tile([128, NSH], DT, tag=f"q{oh}", name=f"q{oh}")
            for oh in range(2)]
    for oh in range(2):
        for ch in range(2):
            qp = ps512.tile([128, 512], f32, tag="mm512", name="mm512")
            for t in range(2):
                nc.tensor.matmul(
                    qp[:],
                    w_sb[0][t][:, oh * 128:(oh + 1) * 128],
                    xqr[t][:, ch * 512:(ch + 1) * 512],
                    start=(t == 0), stop=(t == 1),
                )
            nc.scalar.activation(
                q_sb[oh][:, ch * 512:(ch + 1) * 512], qp[:],
                AF.Identity, bias=beff[0][oh][:],
            )

    # ---- k = w1' @ x + beff1 : [C(2 tiles), N] ----
    k_sb = [pb.tile([128, N], DT, tag=f"k{oh}", name=f"k{oh}")
            for oh in range(2)]
    for oh in range(2):
        for ch in range(NCH):
            kp = ps512.tile([128, 512], f32, tag="mm512", name="mm512")
            for t in range(2):
                nc.tensor.matmul(
                    kp[:],
                    w_sb[1][t][:, oh * 128:(oh + 1) * 128],
                    xs[t][:, ch * 512:(ch + 1) * 512],
                    start=(t == 0), stop=(t == 1),
                )
            nc.scalar.activation(
                k_sb[oh][:, ch * 512:(ch + 1) * 512], kp[:],
                AF.Identity, bias=beff[1][oh][:],
            )

    # ---- v^T (+ ones column) : [m (32 tiles of 128), 257] ----
    # vt[m, c] = sum_ci x[ci, m] * w2'[ci, c]   (bias b2 folded into b3eff)
    MT = N // 128  # 32
    VW = C + 2     # 258: cols 256/257 are all-ones (denominator; 257 pads the
    #                fp32r matmul dst to an even free size)
    vt = pb.tile([128, MT * VW], bf16, tag="vt", name="vt")
    vt3 = vt[:].rearrange("p (m w) -> p m w", w=VW)
    nc.vector.tensor_copy(
        vt3[:, :, C:VW],
        ones2[:].rearrange("p (a w) -> p a w", a=1).to_broadcast((128, MT, 2)),
    )

    def emit_vt(mt):
        vp = ps256.tile([128, C], f32, tag="p256", name="p256")
        for t in range(2):
            nc.tensor.matmul(
                vp[:],
                xs[t][:, mt * 128:(mt + 1) * 128],
                w_sb[2][t],
                start=(t == 0), stop=(t == 1),
            )
        nc.vector.tensor_copy(vt[:, mt * VW:mt * VW + C], vp[:])

    if PHASE == 2:
        for mt in range(MT):
            emit_vt(mt)
        _early_out(q_sb)
        ctx.close()
        return

    # ---- attention: S_T -> exp -> AV (flash-style over m tiles) ----
    # out (this shard): OT[n, c] = sum_m E[m, n] * vt[m, c], denominator in
    # col 256.  n = 1024 processed in two 512-halves (PSUM budget), with the
    # second half's S/exp prologue emitted before the first half's epilogue
    # so the PE stream never drains.
    o_sb = [pb.tile([128, NSH], DT, tag=f"o{t}", name=f"o{t}")
            for t in range(2)]
    PIPE = 5
    es = {}
    ots = {}

    def emit_s(half, mt):
        # S psum chunks alternate between the two 2-deep pools => 4-deep
        # rotation, letting S/exp run PIPE iterations ahead of AV.
        pool = ps512 if mt % 2 == 0 else ps256
        tag = "mm512" if mt % 2 == 0 else "p256"
        sp = pool.tile([128, 512], f32, tag=tag, name=tag)
        for t in range(2):
            nc.tensor.matmul(
                sp[:],
                k_sb[t][:, mt * 128:(mt + 1) * 128],
                q_sb[t][:, half * 512:(half + 1) * 512],
                start=(t == 0), stop=(t == 1),
            )
        e = pw.tile([128, 512], bf16, tag="e", name="e", bufs=PIPE + 2)
        nc.scalar.activation(e[:], sp[:], AF.Exp, scale=SCALE)
        es[(half, mt)] = e

    def emit_av(half, mt):
        e = es.pop((half, mt))
        ot = ots[half]
        for ns in range(4):
            nc.tensor.matmul(
                ot[ns][:],
                e[:, ns * 128:(ns + 1) * 128],
                vt[:, mt * VW:(mt + 1) * VW],
                start=(mt == 0), stop=(mt == MT - 1),
            )

    def emit_finish(half):
        # normalize by the ones-column denominator + transpose back to [c, n]
        ot = ots.pop(half)
        for ns in range(4):
            rec = ptiny.tile([128, 1], f32, tag="rec", name="rec")
            nc.vector.reciprocal(rec[:], ot[ns][:, C:C + 1])
            on = pw.tile([128, C], f32, tag="on", name="on")
            nc.scalar.mul(on[:], ot[ns][:, 0:C], rec[:])
            for t in range(2):
                trp = ps256.tile([128, 128], f32, tag="p256", name="p256")
                nc.tensor.transpose(trp[:], on[:, t * 128:(t + 1) * 128], ident)
                nc.vector.tensor_copy(
                    o_sb[t][:, half * 512 + ns * 128:half * 512 + (ns + 1) * 128],
                    trp[:],
                )

    yts = [pw.tile([128, NSH], f32, tag="yt", name="yt", bufs=2)
           for _ in range(2)]

    def emit_nin(ch):
        # out2 = w3 @ O + b3eff ; y = x + out2   (one 512-wide n-chunk)
        for oh in range(2):
            op = ps512.tile([128, 512], f32, tag="mm512", name="mm512")
            for t in range(2):
                nc.tensor.matmul(
                    op[:],
                    w_sb[3][t][:, oh * 128:(oh + 1) * 128],
                    o_sb[t][:, ch * 512:(ch + 1) * 512],
                    start=(t == 0), stop=(t == 1),
                )
            nc.vector.scalar_tensor_tensor(
                yts[oh][:, ch * 512:(ch + 1) * 512], op[:], b3eff[oh][:],
                xq[oh][:, ch * 512:(ch + 1) * 512],
                op0=ALU.add, op1=ALU.add,
            )
            nc.sync.dma_start(
                d["y"][oh * 128:(oh + 1) * 128, ch * 512:(ch + 1) * 512],
                yts[oh][:, ch * 512:(ch + 1) * 512],
            )

    for mt in range(MT):
        emit_vt(mt)
    ots[0] = [pot.tile([128, VW], f32, tag="ot", name="ot") for _ in range(4)]
    for mt in range(PIPE):
        emit_s(0, mt)
    for mt in range(MT):
        if mt + PIPE < MT:
            emit_s(0, mt + PIPE)
        emit_av(0, mt)
    for mt in range(PIPE):
        emit_s(1, mt)
    ots[1] = [pot.tile([128, VW], f32, tag="ot", name="ot") for _ in range(4)]
    emit_finish(0)
    emit_nin(0)
    for mt in range(MT):
        if mt + PIPE < MT:
            emit_s(1, mt + PIPE)
        emit_av(1, mt)
    emit_finish(1)

    if PHASE == 3:
        _early_out(o_sb)
        ctx.close()
        return

    emit_nin(1)



    ctx.close()


_CACHE = {}


def _get_program():
    if "nc" in _CACHE:
        return _CACHE["nc"], _CACHE["dram"]
    nc = bacc.Bacc("TRN2", target_bir_lowering=False, debug=False,
                   enable_asserts=False, num_devices=NCORES)
    d = {}
    d["x"] = nc.dram_tensor("x", [C, N], bf16, kind="ExternalInput").ap()
    d["xq"] = nc.dram_tensor("xq", [C, NSH], f32, kind="ExternalInput").ap()
    d["wbt"] = nc.dram_tensor("wbt", [C, 3 * C], bf16, kind="ExternalInput").ap()
    d["w3t"] = nc.dram_tensor("w3t", [C, C], DT, kind="ExternalInput").ap()
    d["cpack"] = nc.dram_tensor("cpack", [128, 288], f32,
                                kind="ExternalInput").ap()
    d["y"] = nc.dram_tensor("y", [C, NSH], f32, kind="ExternalOutput").ap()

    with tile.TileContext(nc) as tc:
        _build_body(nc, tc, d)
    nc.compile()
    _CACHE["nc"] = nc
    _CACHE["dram"] = d
    return nc, d


def make_in_maps(x, gamma, beta, w0, b0, w1, b1, w2, b2, w3, b3):
    """Host-side sharding: returns list of 8 per-core input dicts."""
    xb = np.ascontiguousarray(np.asarray(x, np.float32).reshape(B, C, N))
    cvec = np.zeros((C, 8), np.float32)
    for col, v in enumerate([gamma, beta, b0, b1, b2, b3]):
        cvec[:, col] = np.asarray(v, np.float32)
    cvec[:, 6] = -cvec[:, 0]  # -gamma, for the fused bfold computation
    gmask = np.zeros((128, 16), np.float32)
    gmask[np.arange(128), np.arange(128) // CPG] = 1.0
    gmaskT = np.ascontiguousarray(gmask.T)
    gmask = gmask * np.float32(1.0 / NPG)  # fold the 1/NPG of mean/E[x^2]
    ident = np.eye(128, dtype=np.float32)
    cpack = np.zeros((128, 288), np.float32)
    cpack[:, 0:8] = cvec[0:128]
    cpack[:, 8:16] = cvec[128:256]
    cpack[:, 16:32] = gmask
    cpack[0:16, 32:160] = gmaskT
    cpack[:, 160:288] = ident
    wts = [np.ascontiguousarray(np.asarray(w, np.float32).T)
           for w in (w0, w1, w2, w3)]
    wbt = np.concatenate(wts[:3], axis=1).astype(ml_dtypes.bfloat16)
    w3t = wts[3]
    in_maps = []
    for core in range(NCORES):
        b, j = divmod(core, 4)
        m = {
            "x": xb[b].astype(ml_dtypes.bfloat16),
            "xq": np.ascontiguousarray(xb[b][:, j * NSH:(j + 1) * NSH]),
            "wbt": wbt, "w3t": w3t, "cpack": cpack,
        }
        in_maps.append(m)
    return in_maps


def assemble_output(results):
    """results: list of 8 dicts with 'y' [C, NSH] -> full [B,C,16,16,16]."""
    out = np.zeros((B, C, N), np.float32)
    for core in range(NCORES):
        b, j = divmod(core, 4)
        out[b][:, j * NSH:(j + 1) * NSH] = results[core]["y"]
    return out.reshape(B, C, 16, 16, 16)


def kernel(x, gamma, beta, w0, b0, w1, b1, w2, b2, w3, b3):
    nc, _ = _get_program()
    in_maps = make_in_maps(x, gamma, beta, w0, b0, w1, b1, w2, b2, w3, b3)
    res = bass_utils.run_bass_kernel_spmd(nc, in_maps, core_ids=list(range(NCORES)))
    return assemble_output(res.results)



# revision 12
# speedup vs baseline: 1.5707x; 1.5707x over previous
"""Trainium2 Bass/Tile kernel for AttnBlock:
GroupNorm(32) -> 1x1 conv q,k,v -> softmax attention over N=4096 tokens
-> 1x1 conv proj -> residual.

Sharding: 8 cores = 2 (batch) x 4 (query-token shards of N).  Each core gets
the full x of its batch plus its n-shard slice, and produces the [C, N/4]
output shard.  No collectives.

Architecture (v4):
- All heavy matmuls are fp8 MatmulPerfMode.DoubleRow: the full K=256
  contraction in one instruction at 0.5 cycles/output-column.  DR stationary
  operands need their 256 weight elements contiguous per partition; every
  lhsT is laid out [.., kt(2), 128].
- GroupNorm stats via a PE Gram-matrix over the m-major fp8 x copy
  (diag -> sum x^2, ones-matmul -> sum x), diag extracted by one DVE
  scalar_tensor_tensor+accum per c-tile.
- No k tensor: S^T = x^T g with g = a*(w1^T q) [C, NSH] -- the PSUM->SBUF
  drain is the n-shard-sized g (2K lanes) instead of the m-sized k (8K).
  The k bias is dropped exactly (softmax shift invariance); q keeps its
  effective bias.
- No v tensor: attention accumulates over x itself:
  AVx[n, c] = sum_m E[m, n] x[c, m] (moving operand = resident xTw8),
  plus denominator columns from a tiny ones matmul per ns.  After
  normalize + transpose, ONE DoubleRow projection by w238 = a*(w3 w2)^T
  (host-folded w3@w2, scaled 2^19 for fp8) produces the output; the scale
  is undone in the final scalar_tensor_tensor against xqb = x + b3eff.
- Softmax over 2-bank [128,1024] S^T psum tiles; exp ns-subtiles split
  between ACT (true Exp -> fp8e5) and DVE (Schraudolph bits =
  round(logit*4*log2e + 60) as uint8 == fp8e5m2; e5m2 because logits span
  +-8).  Output APs are permuted so E tiles come out [ns, kt, j] -- the
  DR lhsT layout for AVx.
"""

import ml_dtypes
import numpy as np

import concourse.bacc as bacc
import concourse.mybir as mybir
import concourse.tile as tile
from concourse import bass_utils

f32 = mybir.dt.float32
bf16 = mybir.dt.bfloat16
fp8 = mybir.dt.float8e4
fp8e5 = mybir.dt.float8e5
u8 = mybir.dt.uint8
AF = mybir.ActivationFunctionType
ALU = mybir.AluOpType
DR = mybir.MatmulPerfMode.DoubleRow

B = 2
C = 256
N = 4096          # 16**3 tokens
NSH = N // 4      # 1024 tokens per core
G = 32
CPG = C // G      # channels per group
NPG = CPG * N     # elements per group
EPS = 1e-6
SCALE = C ** -0.5          # 1/16
LOG2E = float(1.0 / np.log(2.0))
WS = 524288.0              # 2^19 fp8-range scale on w23; undone in the stt
MT = N // 128              # 32 m-tiles

NCORES = 8

# cpack column layout
CV0, CV1 = 0, 8            # cvec slice0/1: [gamma, beta, b0, bout, -gamma]
GMA, GMB = 16, 48          # gmask per slice [128, 32] (1/NPG folded)
GTA, GTB = 80, 208         # gmaskT per slice [32, 128] on partitions 0:32
MZL = 336                  # zeros[128] | ident[128] | zeros[128]
IDT = 464
EPC = 720                  # eps column
CPW = 728

# engine splits (True -> ACT, False -> DVE)
EXP_SPLIT = [2] * 32       # of 4 ns-subtiles per (half*16+pair), how many ACT
GEP_ACT = [True, False]    # g drain per c-slice
ON_ACT = [True] * 8        # normalize per (half*4 + ns)
TRP_ACT = [True, False, True, False]  # attnx drain per (half*2 + t)

N_WARMUP = 42
PHASE = 4


def _build_body(nc, tc, d):
    from contextlib import ExitStack

    ctx = ExitStack()
    pc = ctx.enter_context(tc.tile_pool(name="const", bufs=1))
    pb = ctx.enter_context(tc.tile_pool(name="big", bufs=1))
    pw = ctx.enter_context(tc.tile_pool(name="work", bufs=3))
    ptiny = ctx.enter_context(tc.tile_pool(name="tiny", bufs=2))
    # PSUM: sp = 2 x [128,1024] (2 banks each), ot = 4 x [128,512] (1 bank)
    sp = ctx.enter_context(tc.tile_pool(name="sp", bufs=2, space="PSUM"))
    ot = ctx.enter_context(tc.tile_pool(name="pot", bufs=4, space="PSUM"))

    # ---- tiny consts ----
    zcol = pc.tile([128, 1], f32, tag="zcol", name="zcol")
    nc.vector.memset(zcol[:], 0.0)
    nc.const_aps.aps[(f32, 0.0)] = zcol[:]
    ones4 = pc.tile([128, 2, 1], fp8, tag="ones4", name="ones4")
    nc.vector.memset(ones4[:], 1.0)
    ones5 = pc.tile([128, 2, 1], fp8e5, tag="ones5", name="ones5")
    nc.vector.memset(ones5[:], 1.0)

    # ---- PE warmup: dep-free matmuls bridge the DMA head + pstate ramp
    wdum = pc.tile([128, 128], bf16, tag="wdum", name="wdum")
    nc.vector.memset(wdum[:], 1.0)
    wslot = ot.tile([128, 512], f32, tag="warm", name="warm")
    for i in range(N_WARMUP):
        nc.tensor.matmul(wslot[:, 0:128], wdum[:], wdum[:],
                         start=True, stop=True)

    # ---- input DMAs: the DMA fabric is serial -- order by need.
    # xTw8[p, pr, t, kt, j] = x[t*128+j, (2*pr+kt)*128+p], in quarters
    xTw8 = pb.tile([128, 16, 2, 2, 128], fp8, tag="xTw8", name="xTw8")
    xTw8f = xTw8[:].rearrange("p a b c e -> p (a b c e)")
    qs = [nc.sync, nc.scalar]
    cpack = pc.tile([128, CPW], f32, tag="cpack", name="cpack")
    for qr in range(4):
        qs[qr % 2].dma_start(xTw8f[:, qr * 2048:(qr + 1) * 2048],
                             d["xTw8"][:, qr * 2048:(qr + 1) * 2048])
        if qr == 1:
            nc.sync.dma_start(cpack[:], d["cpack"][:])
    # xq8[p, kt, n] = x[kt*128+p, shard + n]
    xq8 = pb.tile([128, 2, NSH], fp8, tag="xq8", name="xq8")
    nc.scalar.dma_start(xq8[:].rearrange("p a b -> p (a b)"), d["xq8"][:])
    # wb[p, oh, kt, j] = w0^T[kt*128+p, oh*128+j]
    wb = pb.tile([128, 2, 2, 128], bf16, tag="wb", name="wb")
    nc.sync.dma_start(wb[:].rearrange("p a b c -> p (a b c)"), d["wb"][:])
    # w1p8[p, cs, kto, j] = w1[kto*128+p, cs*128+j]  (plain w1, fp8)
    w1p8 = pb.tile([128, 2, 2, 128], fp8, tag="w1p8", name="w1p8")
    nc.scalar.dma_start(w1p8[:].rearrange("p a b c -> p (a b c)"), d["w1p8"][:])
    # w23t[p, oh, kt, j] = (w3 w2)^T[kt*128+p, oh*128+j]
    w23t = pb.tile([128, 2, 2, 128], bf16, tag="w23t", name="w23t")
    nc.sync.dma_start(w23t[:].rearrange("p a b c -> p (a b c)"), d["w23t"][:])
    # xs8[p, mt, kt, j] = x[kt*128+p, mt*128+j], halves
    xs8 = pb.tile([128, MT, 2, 128], fp8, tag="xs8", name="xs8")
    xs8f = xs8[:].rearrange("p a b c -> p (a b c)")
    nc.scalar.dma_start(xs8f[:, 0:4096], d["xs8"][:, 0:4096])
    nc.sync.dma_start(xs8f[:, 4096:8192], d["xs8"][:, 4096:8192])
    # xq (f32 residual) is emitted LAST -- only needed by the final stt

    cvec = [cpack[:, CV0:CV0 + 8], cpack[:, CV1:CV1 + 8]]
    gm = [cpack[:, GMA:GMA + 32], cpack[:, GMB:GMB + 32]]
    gmt = [cpack[0:32, GTA:GTA + 128], cpack[0:32, GTB:GTB + 128]]
    ident = cpack[:, IDT:IDT + 128]
    dmask = [cpack[:, IDT:IDT + 256], cpack[:, MZL:MZL + 256]]
    epscol = cpack[0:32, EPC:EPC + 1]

    # ---- GroupNorm stats via PE Gram over xTw8 ----
    # gtile: [t0 gram 0:256 | t0 sum-x 256 | pad | t1 gram 512:768 | t1 sum-x]
    gtile = sp.tile([128, 1024], f32, tag="sp", name="gram")
    goff = [0, 512]
    for t in range(2):
        for pr in range(16):
            lhs = xTw8[:, pr, t]
            nc.tensor.matmul(
                gtile[:, goff[t]:goff[t] + 256],
                lhs,
                xTw8[:, pr].rearrange("p t kt j -> p kt t j"),
                start=(pr == 0), stop=(pr == 15), perf_mode=DR,
            )
            nc.tensor.matmul(
                gtile[:, goff[t] + 256:goff[t] + 257],
                lhs, ones4[:],
                start=(pr == 0), stop=(pr == 15), perf_mode=DR,
            )
    # pt[t]: col0 = sum x, col1 = sum x^2 (diag extract)
    pt = [ptiny.tile([128, 2], f32, tag=f"pt{t}", name=f"pt{t}") for t in range(2)]
    trash = pw.tile([128, 256], f32, tag="trash", name="trash", bufs=2)
    for t in range(2):
        nc.vector.tensor_copy(pt[t][:, 0:1], gtile[:, goff[t] + 256:goff[t] + 257])
        nc.vector.scalar_tensor_tensor(
            trash[:], gtile[:, goff[t]:goff[t] + 256], 1.0, dmask[t],
            op0=ALU.mult, op1=ALU.mult, accum_out=pt[t][:, 1:2],
        )

    # group stats -> mu/rstd -> per-channel a, bfold
    s32 = ot.tile([128, 512], f32, tag="warm", name="s32")
    for t in range(2):
        nc.tensor.matmul(s32[0:32, 0:2], gm[t], pt[t][:],
                         start=(t == 0), stop=(t == 1))
    sg = ptiny.tile([32, 2], f32, tag="sg", name="sg")
    nc.vector.tensor_copy(sg[:], s32[0:32, 0:2])
    mr = ptiny.tile([32, 2], f32, tag="mr", name="mr")
    musq = ptiny.tile([32, 1], f32, tag="musq", name="musq")
    nc.vector.tensor_mul(musq[:], sg[:, 0:1], sg[:, 0:1])
    var = ptiny.tile([32, 1], f32, tag="var", name="var")
    nc.vector.tensor_sub(var[:], sg[:, 1:2], musq[:])
    std = ptiny.tile([32, 1], f32, tag="std", name="std")
    nc.scalar.activation(std[:], var[:], AF.Sqrt, bias=epscol)
    nc.vector.reciprocal(mr[:, 1:2], std[:])
    nc.vector.tensor_mul(mr[:, 0:1], sg[:, 0:1], mr[:, 1:2])

    a_t, bfold_bf = [], []
    bc = ot.tile([128, 512], f32, tag="warm", name="bc")
    for t in range(2):
        nc.tensor.matmul(bc[:, 2 * t:2 * t + 2], gmt[t], mr[:],
                         start=True, stop=True)
    for t in range(2):
        a = pb.tile([128, 1], f32, tag=f"a{t}", name=f"a{t}")
        nc.vector.tensor_mul(a[:], bc[:, 2 * t + 1:2 * t + 2], cvec[t][:, 0:1])
        bf = pb.tile([128, 1], f32, tag=f"bf{t}", name=f"bf{t}")
        nc.vector.tensor_scalar(
            bf[:], bc[:, 2 * t:2 * t + 1], cvec[t][:, 4:5], cvec[t][:, 1:2],
            op0=ALU.mult, op1=ALU.add,
        )
        bb = pb.tile([128, 1], bf16, tag=f"bfb{t}", name=f"bfb{t}")
        nc.vector.tensor_copy(bb[:], bf[:])
        a_t.append(a)
        bfold_bf.append(bb)

    def _early_out(srcs):
        for oh in range(2):
            for ch in range(2):
                yt = pw.tile([128, 512], f32, tag="yt", name="yt")
                nc.vector.tensor_copy(
                    yt[:], srcs[oh][:, ch * 512:(ch + 1) * 512])
                nc.sync.dma_start(
                    d["y"][:, oh * NSH + ch * 512:oh * NSH + (ch + 1) * 512],
                    yt[:])

    # ---- effective biases (RAW weights -- emitted before the a-fold) ----
    beff0 = []
    for oh in range(2):
        bp = ot.tile([128, 512], f32, tag="warm", name="bp")
        for t in range(2):
            nc.tensor.matmul(bp[:, 0:1], wb[:, oh, t],
                             bfold_bf[t][:], start=(t == 0), stop=(t == 1))
        bs = pb.tile([128, 1], f32, tag=f"beff0_{oh}", name=f"beff0_{oh}")
        nc.scalar.activation(bs[:], bp[:, 0:1], AF.Identity,
                             bias=cvec[oh][:, 2:3])
        beff0.append(bs)
    b3eff = []
    for oh in range(2):
        bp = ot.tile([128, 512], f32, tag="warm", name="bp3")
        for t in range(2):
            nc.tensor.matmul(bp[:, 0:1], w23t[:, oh, t],
                             bfold_bf[t][:], start=(t == 0), stop=(t == 1))
        bs = pb.tile([128, 1], f32, tag=f"b3eff{oh}", name=f"b3eff{oh}")
        nc.scalar.activation(bs[:], bp[:, 0:1], AF.Identity,
                             bias=cvec[oh][:, 3:4])
        b3eff.append(bs)

    # xqb = x-shard + b3eff (f32); the final stt adds proj*2^-19 onto it.
    # xq is the last input DMA issued -- its transfer queues behind all the
    # early-needed inputs on the serial DMA fabric but lands well before the
    # first finish phase needs xqb.
    xq = pb.tile([128, 2, NSH], f32, tag="xq", name="xq")
    nc.sync.dma_start(xq[:].rearrange("p a b -> p (a b)"), d["xq"][:])
    xqb = pb.tile([128, 2, NSH], f32, tag="xqb", name="xqb")
    for t in range(2):
        nc.vector.tensor_scalar(xqb[:, t], xq[:, t], b3eff[t][:], None,
                                op0=ALU.add)

    # ---- fold a into w0 (in place, bf16) -> fp8; w238 = a*(w3 w2)^T * WS ----
    w018 = pb.tile([128, 2, 2, 128], fp8, tag="w018", name="w018")
    w23s = pb.tile([128, 2, 2, 128], bf16, tag="w23s", name="w23s")
    w238 = pb.tile([128, 2, 2, 128], fp8, tag="w238", name="w238")
    for t in range(2):
        nc.vector.tensor_scalar_mul(wb[:, :, t], wb[:, :, t], a_t[t][:])
        nc.gpsimd.tensor_copy(w018[:, :, t], wb[:, :, t])
        nc.vector.tensor_scalar(w23s[:, :, t], w23t[:, :, t], a_t[t][:], WS,
                                op0=ALU.mult, op1=ALU.mult)
        nc.gpsimd.tensor_copy(w238[:, :, t], w23s[:, :, t])

    if PHASE <= 1:
        _early_out([xq[:, 0], xq[:, 1]])
        ctx.close()
        return

    # ---- q = w0a @ xq + beff0 : fp8 [128, 2(kt=oh), NSH] ----
    q2 = pb.tile([128, 2, NSH], fp8, tag="q2", name="q2")
    for oh in range(2):
        qp = sp.tile([128, 1024], f32, tag="sp", name="qp")
        for ch in range(2):
            nc.tensor.matmul(
                qp[:, ch * 512:(ch + 1) * 512],
                w018[:, oh],
                xq8[:, :, ch * 512:(ch + 1) * 512],
                start=True, stop=True, perf_mode=DR,
            )
        nc.scalar.activation(q2[:, oh, :], qp[:], AF.Identity,
                             bias=beff0[oh][:])

    # ---- g = a * (w1^T q) : fp8 [128, 2(kt=c-slice), NSH] ----
    g8 = pb.tile([128, 2, NSH], fp8, tag="g8", name="g8")
    for cs in range(2):
        gp = sp.tile([128, 1024], f32, tag="sp", name="gp")
        for h in range(2):
            nc.tensor.matmul(
                gp[:, h * 512:(h + 1) * 512],
                w1p8[:, cs],
                q2[:, :, h * 512:(h + 1) * 512],
                start=True, stop=True, perf_mode=DR,
            )
        if GEP_ACT[cs]:
            nc.scalar.activation(g8[:, cs, :], gp[:], AF.Copy,
                                 scale=a_t[cs][:])
        else:
            nc.vector.tensor_scalar_mul(g8[:, cs, :], gp[:], a_t[cs][:])

    if PHASE == 2:
        _early_out([xq[:, 0], xq[:, 1]])
        ctx.close()
        return

    # ---- attention ----
    yts = [pb.tile([128, NSH], f32, tag=f"yts{t}", name=f"yts{t}")
           for t in range(2)]
    # attnx8[p, t(kt for proj), half, n] -- normalized attention-averaged x
    attnx8 = pb.tile([128, 2, 2, 512], fp8, tag="attnx8", name="attnx8")

    for half in range(2):
        ots = [ot.tile([128, 512], f32, tag="warm", name=f"ot{half}_{ns}")
               for ns in range(4)]
        for pr in range(16):
            # S^T pair: S[m, n] = sum_c x[c, m] g[c, n]
            st = sp.tile([128, 1024], f32, tag="sp", name="st")
            for h in range(2):
                nc.tensor.matmul(
                    st[:, h * 512:(h + 1) * 512],
                    xs8[:, 2 * pr + h],
                    g8[:, :, half * 512:(half + 1) * 512],
                    start=True, stop=True, perf_mode=DR,
                )
            # exp -> fp8e5 E chunk, permuted out to [p, ns, kt(2 mt), j]
            e = pw.tile([128, 4, 2, 128], u8, tag="e", name="e", bufs=5)
            stv = st[:].rearrange("p (kt ns j) -> p kt ns j", kt=2, ns=4)
            ev = e[:].rearrange("p ns kt j -> p kt ns j")
            a = EXP_SPLIT[half * 16 + pr]
            if a > 0:
                nc.scalar.activation(ev.bitcast(fp8e5)[:, :, 0:a, :],
                                     stv[:, :, 0:a, :], AF.Exp, scale=SCALE)
            if a < 4:
                nc.vector.tensor_scalar(ev[:, :, a:4, :], stv[:, :, a:4, :],
                                        SCALE * 4.0 * LOG2E, 60.0,
                                        op0=ALU.mult, op1=ALU.add)
            # AVx accumulation + denominator columns (in ots[0] cols 256:260)
            xr = xTw8[:, pr].rearrange("p t kt j -> p kt t j")
            for ns in range(4):
                el = e[:, ns].bitcast(fp8e5)
                nc.tensor.matmul(
                    ots[ns][:, 0:256], el, xr,
                    start=(pr == 0), stop=(pr == 15), perf_mode=DR,
                )
                nc.tensor.matmul(
                    ots[0][:, 256 + ns:257 + ns], el, ones5[:],
                    start=(pr == 0), stop=(pr == 15), perf_mode=DR,
                )
        # finish: normalize, transpose to [c, n], drain fp8, project, +xqb
        rec4 = ptiny.tile([128, 4], f32, tag="rec4", name="rec4")
        nc.vector.reciprocal(rec4[:], ots[0][:, 256:260])
        ons = []
        for ns in range(4):
            on = pw.tile([128, 256], f32, tag="on", name="on", bufs=5)
            if ON_ACT[half * 4 + ns]:
                nc.scalar.activation(on[:], ots[ns][:, 0:256], AF.Identity,
                                     scale=rec4[:, ns:ns + 1])
            else:
                nc.vector.tensor_scalar_mul(on[:], ots[ns][:, 0:256],
                                            rec4[:, ns:ns + 1])
            ons.append(on)
        for t in range(2):
            trp = ot.tile([128, 512], f32, tag="warm", name=f"trp{half}_{t}")
            for ns in range(4):
                nc.tensor.transpose(trp[:, ns * 128:(ns + 1) * 128],
                                    ons[ns][:, t * 128:(t + 1) * 128], ident)
            dst = attnx8[:, t, half]
            if TRP_ACT[half * 2 + t]:
                nc.scalar.activation(dst, trp[:], AF.Copy)
            else:
                nc.vector.tensor_copy(dst, trp[:])
        for oh in range(2):
            pj = sp.tile([128, 1024], f32, tag="sp", name="pj")
            nc.tensor.matmul(
                pj[:, 0:512],
                w238[:, oh],
                attnx8[:, :, half],
                start=True, stop=True, perf_mode=DR,
            )
            nc.vector.scalar_tensor_tensor(
                yts[oh][:, half * 512:(half + 1) * 512], pj[:, 0:512],
                1.0 / WS, xqb[:, oh, half * 512:(half + 1) * 512],
                op0=ALU.mult, op1=ALU.add,
            )
            dq = (nc.sync, nc.scalar)[(half + oh) % 2]
            dq.dma_start(
                d["y"][:, oh * NSH + half * 512:oh * NSH + (half + 1) * 512],
                yts[oh][:, half * 512:(half + 1) * 512],
            )

    ctx.close()


_CACHE = {}


def _get_program():
    if "nc" in _CACHE:
        return _CACHE["nc"], _CACHE["dram"]
    nc = bacc.Bacc("TRN2", target_bir_lowering=False, debug=False,
                   enable_asserts=False, num_devices=NCORES)
    d = {}
    d["xs8"] = nc.dram_tensor("xs8", [128, MT * 256], fp8,
                              kind="ExternalInput").ap()
    d["xTw8"] = nc.dram_tensor("xTw8", [128, 16 * 512], fp8,
                               kind="ExternalInput").ap()
    d["xq8"] = nc.dram_tensor("xq8", [128, 2 * NSH], fp8,
                              kind="ExternalInput").ap()
    d["xq"] = nc.dram_tensor("xq", [128, 2 * NSH], f32, kind="ExternalInput").ap()
    d["wb"] = nc.dram_tensor("wb", [128, 4 * 128], bf16, kind="ExternalInput").ap()
    d["w1p8"] = nc.dram_tensor("w1p8", [128, 4 * 128], fp8,
                               kind="ExternalInput").ap()
    d["w23t"] = nc.dram_tensor("w23t", [128, 4 * 128], bf16,
                               kind="ExternalInput").ap()
    d["cpack"] = nc.dram_tensor("cpack", [128, CPW], f32,
                                kind="ExternalInput").ap()
    d["y"] = nc.dram_tensor("y", [128, 2 * NSH], f32, kind="ExternalOutput").ap()

    with tile.TileContext(nc) as tc:
        _build_body(nc, tc, d)
    nc.compile()
    _CACHE["nc"] = nc
    _CACHE["dram"] = d
    return nc, d


def make_in_maps(x, gamma, beta, w0, b0, w1, b1, w2, b2, w3, b3):
    """Host-side sharding/layout prep: returns list of 8 per-core inputs."""
    e4 = ml_dtypes.float8_e4m3
    xb = np.ascontiguousarray(np.asarray(x, np.float32).reshape(B, C, N))

    cpack = np.zeros((128, CPW), np.float32)
    gamma = np.asarray(gamma, np.float32)
    beta = np.asarray(beta, np.float32)
    b0 = np.asarray(b0, np.float32)
    bout = (np.asarray(w3, np.float32) @ np.asarray(b2, np.float32)
            + np.asarray(b3, np.float32))
    for t, off in ((0, CV0), (1, CV1)):
        sl = slice(t * 128, (t + 1) * 128)
        cpack[:, off + 0] = gamma[sl]
        cpack[:, off + 1] = beta[sl]
        cpack[:, off + 2] = b0[sl]
        cpack[:, off + 3] = bout[sl]
        cpack[:, off + 4] = -gamma[sl]
    for t, off in ((0, GMA), (1, GMB)):
        ch = t * 128 + np.arange(128)
        cpack[np.arange(128), off + ch // CPG] = 1.0 / NPG
    for t, off in ((0, GTA), (1, GTB)):
        ch = t * 128 + np.arange(128)
        cpack[ch // CPG, off + np.arange(128)] = 1.0
    cpack[:, IDT:IDT + 128] = np.eye(128, dtype=np.float32)
    cpack[:, EPC] = EPS

    # wb[p, oh, kt, j] = w0^T[kt*128+p, oh*128+j]
    w0t = np.asarray(w0, np.float32).T.reshape(2, 128, 2, 128)  # [kt, p, oh, j]
    wb = w0t.transpose(1, 2, 0, 3).reshape(128, -1).astype(ml_dtypes.bfloat16)
    # w1p8[p, cs, kto, j] = w1[kto*128+p, cs*128+j]
    w1a = np.asarray(w1, np.float32).reshape(2, 128, 2, 128)    # [kto, p, cs, j]
    w1p8 = w1a.transpose(1, 2, 0, 3).reshape(128, -1).astype(e4)
    w23 = (np.asarray(w3, np.float32) @ np.asarray(w2, np.float32)).T
    w23t = w23.reshape(2, 128, 2, 128).transpose(1, 2, 0, 3)
    w23t = w23t.reshape(128, -1).astype(ml_dtypes.bfloat16)

    in_maps = []
    for core in range(NCORES):
        b, j = divmod(core, 4)
        xc = xb[b]
        xs8 = xc.reshape(2, 128, MT, 128).transpose(1, 2, 0, 3)
        xT = xc.reshape(2, 128, 16, 2, 128)  # [t, jj, pr, kt, p]
        xT = xT.transpose(4, 2, 0, 3, 1)
        xqc = xc[:, j * NSH:(j + 1) * NSH]
        xq = xqc.reshape(2, 128, NSH).transpose(1, 0, 2).reshape(128, -1)
        m = {
            "xs8": xs8.reshape(128, -1).astype(e4),
            "xTw8": xT.reshape(128, -1).astype(e4),
            "xq8": xq.astype(e4),
            "xq": np.ascontiguousarray(xq),
            "wb": wb, "w1p8": w1p8, "w23t": w23t, "cpack": cpack,
        }
        in_maps.append(m)
    return in_maps


def assemble_output(results):
    out = np.zeros((B, C, N), np.float32)
    for core in range(NCORES):
        b, j = divmod(core, 4)
        y = results[core]["y"].reshape(128, 2, NSH).transpose(1, 0, 2)
        out[b][:, j * NSH:(j + 1) * NSH] = y.reshape(C, NSH)
    return out.reshape(B, C, 16, 16, 16)


def kernel(x, gamma, beta, w0, b0, w1, b1, w2, b2, w3, b3):
    nc, _ = _get_program()
    in_maps = make_in_maps(x, gamma, beta, w0, b0, w1, b1, w2, b2, w3, b3)
    res = bass_utils.run_bass_kernel_spmd(nc, in_maps, core_ids=list(range(NCORES)))
    return assemble_output(res.results)


# revision 13
# speedup vs baseline: 1.5861x; 1.0098x over previous
"""Trainium2 Bass/Tile kernel for AttnBlock:
GroupNorm(32) -> 1x1 conv q,k,v -> softmax attention over N=4096 tokens
-> 1x1 conv proj -> residual.

Sharding: 8 cores = 2 (batch) x 4 (query-token shards of N).  Each core gets
the full x of its batch plus its n-shard slice, and produces the [C, N/4]
output shard.  No collectives.

Architecture (v4):
- All heavy matmuls are fp8 MatmulPerfMode.DoubleRow: the full K=256
  contraction in one instruction at 0.5 cycles/output-column.  DR stationary
  operands need their 256 weight elements contiguous per partition; every
  lhsT is laid out [.., kt(2), 128].
- GroupNorm stats via a PE Gram-matrix over the m-major fp8 x copy
  (diag -> sum x^2, ones-matmul -> sum x), diag extracted by one DVE
  scalar_tensor_tensor+accum per c-tile.
- No k tensor: S^T = x^T g with g = a*(w1^T q) [C, NSH] -- the PSUM->SBUF
  drain is the n-shard-sized g (2K lanes) instead of the m-sized k (8K).
  The k bias is dropped exactly (softmax shift invariance); q keeps its
  effective bias.
- No v tensor: attention accumulates over x itself:
  AVx[n, c] = sum_m E[m, n] x[c, m] (moving operand = resident xTw8),
  plus denominator columns from a tiny ones matmul per ns.  After
  normalize + transpose, ONE DoubleRow projection by w238 = a*(w3 w2)^T
  (host-folded w3@w2, scaled 2^19 for fp8) produces the output; the scale
  is undone in the final scalar_tensor_tensor against xqb = x + b3eff.
- Softmax over 2-bank [128,1024] S^T psum tiles; exp ns-subtiles split
  between ACT (true Exp -> fp8e5) and DVE (Schraudolph bits =
  round(logit*4*log2e + 60) as uint8 == fp8e5m2; e5m2 because logits span
  +-8).  Output APs are permuted so E tiles come out [ns, kt, j] -- the
  DR lhsT layout for AVx.
"""

import ml_dtypes
import numpy as np

import concourse.bacc as bacc
import concourse.mybir as mybir
import concourse.tile as tile
from concourse import bass_utils

f32 = mybir.dt.float32
bf16 = mybir.dt.bfloat16
fp8 = mybir.dt.float8e4
fp8e5 = mybir.dt.float8e5
u8 = mybir.dt.uint8
AF = mybir.ActivationFunctionType
ALU = mybir.AluOpType
DR = mybir.MatmulPerfMode.DoubleRow

B = 2
C = 256
N = 4096          # 16**3 tokens
NSH = N // 4      # 1024 tokens per core
G = 32
CPG = C // G      # channels per group
NPG = CPG * N     # elements per group
EPS = 1e-6
SCALE = C ** -0.5          # 1/16
LOG2E = float(1.0 / np.log(2.0))
WS = 524288.0              # 2^19 fp8-range scale on w23; undone in the stt
MT = N // 128              # 32 m-tiles

NCORES = 8

# cpack column layout
CV0, CV1 = 0, 8            # cvec slice0/1: [gamma, beta, b0, bout, -gamma]
GMA, GMB = 16, 48          # gmask per slice [128, 32] (1/NPG folded)
GTA, GTB = 80, 208         # gmaskT per slice [32, 128] on partitions 0:32
MZL = 336                  # zeros[128] | ident[128] | zeros[128]
IDT = 464
EPC = 720                  # eps column
CPW = 728

# engine splits (True -> ACT, False -> DVE)
EXP_SPLIT = [2] * 32       # of 4 ns-subtiles per (half*16+pair), how many ACT
GEP_ACT = [True, False]    # g drain per c-slice
ON_ACT = [True] * 8        # normalize per (half*4 + ns)
TRP_ACT = [True, False, True, False]  # attnx drain per (half*2 + t)

N_WARMUP = 42
PHASE = 4


def _build_body(nc, tc, d):
    from contextlib import ExitStack

    ctx = ExitStack()
    pc = ctx.enter_context(tc.tile_pool(name="const", bufs=1))
    pb = ctx.enter_context(tc.tile_pool(name="big", bufs=1))
    pw = ctx.enter_context(tc.tile_pool(name="work", bufs=3))
    ptiny = ctx.enter_context(tc.tile_pool(name="tiny", bufs=2))
    # PSUM: sp = 2 x [128,1024] (2 banks each), ot = 4 x [128,512] (1 bank)
    sp = ctx.enter_context(tc.tile_pool(name="sp", bufs=2, space="PSUM"))
    ot = ctx.enter_context(tc.tile_pool(name="pot", bufs=4, space="PSUM"))

    # ---- tiny consts ----
    zcol = pc.tile([128, 1], f32, tag="zcol", name="zcol")
    nc.vector.memset(zcol[:], 0.0)
    nc.const_aps.aps[(f32, 0.0)] = zcol[:]
    ones4 = pc.tile([128, 2, 1], fp8, tag="ones4", name="ones4")
    nc.vector.memset(ones4[:], 1.0)
    ones5 = pc.tile([128, 2, 1], fp8e5, tag="ones5", name="ones5")
    nc.vector.memset(ones5[:], 1.0)

    # ---- PE warmup: dep-free matmuls bridge the DMA head + pstate ramp
    wdum = pc.tile([128, 128], bf16, tag="wdum", name="wdum")
    nc.vector.memset(wdum[:], 1.0)
    wslot = ot.tile([128, 512], f32, tag="warm", name="warm")
    for i in range(N_WARMUP):
        nc.tensor.matmul(wslot[:, 0:128], wdum[:], wdum[:],
                         start=True, stop=True)

    # ---- input DMAs: the DMA fabric is serial -- order by need.
    # xTw8[p, pr, t, kt, j] = x[t*128+j, (2*pr+kt)*128+p], in quarters
    xTw8 = pb.tile([128, 16, 2, 2, 128], fp8, tag="xTw8", name="xTw8")
    xTw8f = xTw8[:].rearrange("p a b c e -> p (a b c e)")
    qs = [nc.sync, nc.scalar]
    cpack = pc.tile([128, CPW], f32, tag="cpack", name="cpack")
    for qr in range(4):
        qs[qr % 2].dma_start(xTw8f[:, qr * 2048:(qr + 1) * 2048],
                             d["xTw8"][:, qr * 2048:(qr + 1) * 2048])
        if qr == 1:
            nc.sync.dma_start(cpack[:], d["cpack"][:])
    # xq8[p, kt, n] = x[kt*128+p, shard + n]
    xq8 = pb.tile([128, 2, NSH], fp8, tag="xq8", name="xq8")
    nc.scalar.dma_start(xq8[:].rearrange("p a b -> p (a b)"), d["xq8"][:])
    # wb[p, oh, kt, j] = w0^T[kt*128+p, oh*128+j]
    wb = pb.tile([128, 2, 2, 128], bf16, tag="wb", name="wb")
    nc.sync.dma_start(wb[:].rearrange("p a b c -> p (a b c)"), d["wb"][:])
    # w1p8[p, cs, kto, j] = w1[kto*128+p, cs*128+j]  (plain w1, fp8)
    w1p8 = pb.tile([128, 2, 2, 128], fp8, tag="w1p8", name="w1p8")
    nc.scalar.dma_start(w1p8[:].rearrange("p a b c -> p (a b c)"), d["w1p8"][:])
    # w23t[p, oh, kt, j] = (w3 w2)^T[kt*128+p, oh*128+j]
    w23t = pb.tile([128, 2, 2, 128], bf16, tag="w23t", name="w23t")
    nc.sync.dma_start(w23t[:].rearrange("p a b c -> p (a b c)"), d["w23t"][:])
    # xs8[p, mt, kt, j] = x[kt*128+p, mt*128+j], halves
    xs8 = pb.tile([128, MT, 2, 128], fp8, tag="xs8", name="xs8")
    xs8f = xs8[:].rearrange("p a b c -> p (a b c)")
    nc.scalar.dma_start(xs8f[:, 0:4096], d["xs8"][:, 0:4096])
    nc.sync.dma_start(xs8f[:, 4096:8192], d["xs8"][:, 4096:8192])
    # xq (f32 residual) is emitted LAST -- only needed by the final stt

    cvec = [cpack[:, CV0:CV0 + 8], cpack[:, CV1:CV1 + 8]]
    gm = [cpack[:, GMA:GMA + 32], cpack[:, GMB:GMB + 32]]
    gmt = [cpack[0:32, GTA:GTA + 128], cpack[0:32, GTB:GTB + 128]]
    ident = cpack[:, IDT:IDT + 128]
    dmask = [cpack[:, IDT:IDT + 256], cpack[:, MZL:MZL + 256]]
    epscol = cpack[0:32, EPC:EPC + 1]

    # ---- GroupNorm stats via PE Gram over xTw8 ----
    # gtile: [t0 gram 0:256 | t0 sum-x 256 | pad | t1 gram 512:768 | t1 sum-x]
    gtile = sp.tile([128, 1024], f32, tag="sp", name="gram")
    goff = [0, 512]
    for t in range(2):
        for pr in range(16):
            lhs = xTw8[:, pr, t]
            nc.tensor.matmul(
                gtile[:, goff[t]:goff[t] + 256],
                lhs,
                xTw8[:, pr].rearrange("p t kt j -> p kt t j"),
                start=(pr == 0), stop=(pr == 15), perf_mode=DR,
            )
            nc.tensor.matmul(
                gtile[:, goff[t] + 256:goff[t] + 257],
                lhs, ones4[:],
                start=(pr == 0), stop=(pr == 15), perf_mode=DR,
            )
    # pt[t]: col0 = sum x, col1 = sum x^2 (diag extract)
    pt = [ptiny.tile([128, 2], f32, tag=f"pt{t}", name=f"pt{t}") for t in range(2)]
    trash = pw.tile([128, 256], f32, tag="trash", name="trash", bufs=2)
    for t in range(2):
        nc.vector.tensor_copy(pt[t][:, 0:1], gtile[:, goff[t] + 256:goff[t] + 257])
        nc.vector.scalar_tensor_tensor(
            trash[:], gtile[:, goff[t]:goff[t] + 256], 1.0, dmask[t],
            op0=ALU.mult, op1=ALU.mult, accum_out=pt[t][:, 1:2],
        )

    # group stats -> mu/rstd -> per-channel a, bfold
    s32 = ot.tile([128, 512], f32, tag="warm", name="s32")
    for t in range(2):
        nc.tensor.matmul(s32[0:32, 0:2], gm[t], pt[t][:],
                         start=(t == 0), stop=(t == 1))
    sg = ptiny.tile([32, 2], f32, tag="sg", name="sg")
    nc.vector.tensor_copy(sg[:], s32[0:32, 0:2])
    mr = ptiny.tile([32, 2], f32, tag="mr", name="mr")
    musq = ptiny.tile([32, 1], f32, tag="musq", name="musq")
    nc.vector.tensor_mul(musq[:], sg[:, 0:1], sg[:, 0:1])
    var = ptiny.tile([32, 1], f32, tag="var", name="var")
    nc.vector.tensor_sub(var[:], sg[:, 1:2], musq[:])
    std = ptiny.tile([32, 1], f32, tag="std", name="std")
    nc.scalar.activation(std[:], var[:], AF.Sqrt, bias=epscol)
    nc.vector.reciprocal(mr[:, 1:2], std[:])
    nc.vector.tensor_mul(mr[:, 0:1], sg[:, 0:1], mr[:, 1:2])

    a_t, bfold_bf = [], []
    bc = ot.tile([128, 512], f32, tag="warm", name="bc")
    for t in range(2):
        nc.tensor.matmul(bc[:, 2 * t:2 * t + 2], gmt[t], mr[:],
                         start=True, stop=True)
    for t in range(2):
        a = pb.tile([128, 1], f32, tag=f"a{t}", name=f"a{t}")
        nc.vector.tensor_mul(a[:], bc[:, 2 * t + 1:2 * t + 2], cvec[t][:, 0:1])
        bf = pb.tile([128, 1], f32, tag=f"bf{t}", name=f"bf{t}")
        nc.vector.tensor_scalar(
            bf[:], bc[:, 2 * t:2 * t + 1], cvec[t][:, 4:5], cvec[t][:, 1:2],
            op0=ALU.mult, op1=ALU.add,
        )
        bb = pb.tile([128, 1], bf16, tag=f"bfb{t}", name=f"bfb{t}")
        nc.vector.tensor_copy(bb[:], bf[:])
        a_t.append(a)
        bfold_bf.append(bb)

    def _early_out(srcs):
        for oh in range(2):
            for ch in range(2):
                yt = pw.tile([128, 512], f32, tag="yt", name="yt")
                nc.vector.tensor_copy(
                    yt[:], srcs[oh][:, ch * 512:(ch + 1) * 512])
                nc.sync.dma_start(
                    d["y"][:, oh * NSH + ch * 512:oh * NSH + (ch + 1) * 512],
                    yt[:])

    # ---- effective biases (RAW weights -- emitted before the a-fold) ----
    beff0 = []
    for oh in range(2):
        bp = ot.tile([128, 512], f32, tag="warm", name="bp")
        for t in range(2):
            nc.tensor.matmul(bp[:, 0:1], wb[:, oh, t],
                             bfold_bf[t][:], start=(t == 0), stop=(t == 1))
        bs = pb.tile([128, 1], f32, tag=f"beff0_{oh}", name=f"beff0_{oh}")
        nc.scalar.activation(bs[:], bp[:, 0:1], AF.Identity,
                             bias=cvec[oh][:, 2:3])
        beff0.append(bs)
    b3eff = []
    for oh in range(2):
        bp = ot.tile([128, 512], f32, tag="warm", name="bp3")
        for t in range(2):
            nc.tensor.matmul(bp[:, 0:1], w23t[:, oh, t],
                             bfold_bf[t][:], start=(t == 0), stop=(t == 1))
        bs = pb.tile([128, 1], f32, tag=f"b3eff{oh}", name=f"b3eff{oh}")
        nc.scalar.activation(bs[:], bp[:, 0:1], AF.Identity,
                             bias=cvec[oh][:, 3:4])
        b3eff.append(bs)

    # xqb = x-shard + b3eff (f32); the final stt adds proj*2^-19 onto it.
    # xq is the last input DMA issued -- its transfer queues behind all the
    # early-needed inputs on the serial DMA fabric but lands well before the
    # first finish phase needs xqb.
    xq = pb.tile([128, 2, NSH], f32, tag="xq", name="xq")
    nc.sync.dma_start(xq[:].rearrange("p a b -> p (a b)"), d["xq"][:])
    xqb = pb.tile([128, 2, NSH], f32, tag="xqb", name="xqb")
    for t in range(2):
        nc.vector.tensor_scalar(xqb[:, t], xq[:, t], b3eff[t][:], None,
                                op0=ALU.add)

    # ---- fold a into w0 (in place, bf16) -> fp8; w238 = a*(w3 w2)^T * WS ----
    w018 = pb.tile([128, 2, 2, 128], fp8, tag="w018", name="w018")
    w23s = pb.tile([128, 2, 2, 128], bf16, tag="w23s", name="w23s")
    w238 = pb.tile([128, 2, 2, 128], fp8, tag="w238", name="w238")
    for t in range(2):
        nc.vector.tensor_scalar_mul(wb[:, :, t], wb[:, :, t], a_t[t][:])
        nc.gpsimd.tensor_copy(w018[:, :, t], wb[:, :, t])
        nc.vector.tensor_scalar(w23s[:, :, t], w23t[:, :, t], a_t[t][:], WS,
                                op0=ALU.mult, op1=ALU.mult)
        nc.gpsimd.tensor_copy(w238[:, :, t], w23s[:, :, t])

    if PHASE <= 1:
        _early_out([xq[:, 0], xq[:, 1]])
        ctx.close()
        return

    # ---- q = w0a @ xq + beff0 : fp8 [128, 2(kt=oh), NSH] ----
    q2 = pb.tile([128, 2, NSH], fp8, tag="q2", name="q2")
    for oh in range(2):
        qp = sp.tile([128, 1024], f32, tag="sp", name="qp")
        for ch in range(2):
            nc.tensor.matmul(
                qp[:, ch * 512:(ch + 1) * 512],
                w018[:, oh],
                xq8[:, :, ch * 512:(ch + 1) * 512],
                start=True, stop=True, perf_mode=DR,
            )
        nc.scalar.activation(q2[:, oh, :], qp[:], AF.Identity,
                             bias=beff0[oh][:])

    # ---- g = a * (w1^T q) : fp8 [128, 2(kt=c-slice), NSH] ----
    g8 = pb.tile([128, 2, NSH], fp8, tag="g8", name="g8")
    for cs in range(2):
        gp = sp.tile([128, 1024], f32, tag="sp", name="gp")
        for h in range(2):
            nc.tensor.matmul(
                gp[:, h * 512:(h + 1) * 512],
                w1p8[:, cs],
                q2[:, :, h * 512:(h + 1) * 512],
                start=True, stop=True, perf_mode=DR,
            )
        if GEP_ACT[cs]:
            nc.scalar.activation(g8[:, cs, :], gp[:], AF.Copy,
                                 scale=a_t[cs][:])
        else:
            nc.vector.tensor_scalar_mul(g8[:, cs, :], gp[:], a_t[cs][:])

    if PHASE == 2:
        _early_out([xq[:, 0], xq[:, 1]])
        ctx.close()
        return

    # ---- attention ----
    yts = [pb.tile([128, NSH], f32, tag=f"yts{t}", name=f"yts{t}")
           for t in range(2)]
    # attnx8[p, t(kt for proj), half, n] -- normalized attention-averaged x
    attnx8 = pb.tile([128, 2, 2, 512], fp8, tag="attnx8", name="attnx8")

    def emit_s_exp(half, pr):
        # S^T pair: S[m, n] = sum_c x[c, m] g[c, n], then exp -> E chunk
        st = sp.tile([128, 1024], f32, tag="sp", name="st")
        for h in range(2):
            nc.tensor.matmul(
                st[:, h * 512:(h + 1) * 512],
                xs8[:, 2 * pr + h],
                g8[:, :, half * 512:(half + 1) * 512],
                start=True, stop=True, perf_mode=DR,
            )
        # exp -> fp8e5 E chunk, permuted out to [p, ns, kt(2 mt), j]
        e = pw.tile([128, 4, 2, 128], u8, tag="e", name="e", bufs=5)
        stv = st[:].rearrange("p (kt ns j) -> p kt ns j", kt=2, ns=4)
        ev = e[:].rearrange("p ns kt j -> p kt ns j")
        a = EXP_SPLIT[half * 16 + pr]
        if a > 0:
            nc.scalar.activation(ev.bitcast(fp8e5)[:, :, 0:a, :],
                                 stv[:, :, 0:a, :], AF.Exp, scale=SCALE)
        if a < 4:
            nc.vector.tensor_scalar(ev[:, :, a:4, :], stv[:, :, a:4, :],
                                    SCALE * 4.0 * LOG2E, 60.0,
                                    op0=ALU.mult, op1=ALU.add)
        return e

    def emit_avx(half, pr, e, ots):
        # AVx accumulation + denominator columns (in ots[0] cols 256:260)
        xr = xTw8[:, pr].rearrange("p t kt j -> p kt t j")
        for ns in range(4):
            el = e[:, ns].bitcast(fp8e5)
            nc.tensor.matmul(
                ots[ns][:, 0:256], el, xr,
                start=(pr == 0), stop=(pr == 15), perf_mode=DR,
            )
            nc.tensor.matmul(
                ots[0][:, 256 + ns:257 + ns], el, ones5[:],
                start=(pr == 0), stop=(pr == 15), perf_mode=DR,
            )

    for half in range(2):
        ots = [ot.tile([128, 512], f32, tag="warm", name=f"ot{half}_{ns}")
               for ns in range(4)]
        # software pipeline: S/exp of pair p+1 is emitted (and thus ordered
        # on the PE) before AVx of pair p, so the PE never idles behind exp.
        e_cur = emit_s_exp(half, 0)
        for pr in range(16):
            e_nxt = emit_s_exp(half, pr + 1) if pr + 1 < 16 else None
            emit_avx(half, pr, e_cur, ots)
            e_cur = e_nxt
        # finish: normalize, transpose to [c, n], drain fp8, project, +xqb
        rec4 = ptiny.tile([128, 4], f32, tag="rec4", name="rec4")
        nc.vector.reciprocal(rec4[:], ots[0][:, 256:260])
        ons = []
        for ns in range(4):
            on = pw.tile([128, 256], f32, tag="on", name="on", bufs=5)
            if ON_ACT[half * 4 + ns]:
                nc.scalar.activation(on[:], ots[ns][:, 0:256], AF.Identity,
                                     scale=rec4[:, ns:ns + 1])
            else:
                nc.vector.tensor_scalar_mul(on[:], ots[ns][:, 0:256],
                                            rec4[:, ns:ns + 1])
            ons.append(on)
        for t in range(2):
            trp = ot.tile([128, 512], f32, tag="warm", name=f"trp{half}_{t}")
            for ns in range(4):
                nc.tensor.transpose(trp[:, ns * 128:(ns + 1) * 128],
                                    ons[ns][:, t * 128:(t + 1) * 128], ident)
            dst = attnx8[:, t, half]
            if TRP_ACT[half * 2 + t]:
                nc.scalar.activation(dst, trp[:], AF.Copy)
            else:
                nc.vector.tensor_copy(dst, trp[:])
        for oh in range(2):
            pj = sp.tile([128, 1024], f32, tag="sp", name="pj")
            nc.tensor.matmul(
                pj[:, 0:512],
                w238[:, oh],
                attnx8[:, :, half],
                start=True, stop=True, perf_mode=DR,
            )
            nc.vector.scalar_tensor_tensor(
                yts[oh][:, half * 512:(half + 1) * 512], pj[:, 0:512],
                1.0 / WS, xqb[:, oh, half * 512:(half + 1) * 512],
                op0=ALU.mult, op1=ALU.add,
            )
            dq = (nc.sync, nc.scalar)[(half + oh) % 2]
            dq.dma_start(
                d["y"][:, oh * NSH + half * 512:oh * NSH + (half + 1) * 512],
                yts[oh][:, half * 512:(half + 1) * 512],
            )

    ctx.close()


_CACHE = {}


def _get_program():
    if "nc" in _CACHE:
        return _CACHE["nc"], _CACHE["dram"]
    nc = bacc.Bacc("TRN2", target_bir_lowering=False, debug=False,
                   enable_asserts=False, num_devices=NCORES)
    d = {}
    d["xs8"] = nc.dram_tensor("xs8", [128, MT * 256], fp8,
                              kind="ExternalInput").ap()
    d["xTw8"] = nc.dram_tensor("xTw8", [128, 16 * 512], fp8,
                               kind="ExternalInput").ap()
    d["xq8"] = nc.dram_tensor("xq8", [128, 2 * NSH], fp8,
                              kind="ExternalInput").ap()
    d["xq"] = nc.dram_tensor("xq", [128, 2 * NSH], f32, kind="ExternalInput").ap()
    d["wb"] = nc.dram_tensor("wb", [128, 4 * 128], bf16, kind="ExternalInput").ap()
    d["w1p8"] = nc.dram_tensor("w1p8", [128, 4 * 128], fp8,
                               kind="ExternalInput").ap()
    d["w23t"] = nc.dram_tensor("w23t", [128, 4 * 128], bf16,
                               kind="ExternalInput").ap()
    d["cpack"] = nc.dram_tensor("cpack", [128, CPW], f32,
                                kind="ExternalInput").ap()
    d["y"] = nc.dram_tensor("y", [128, 2 * NSH], f32, kind="ExternalOutput").ap()

    with tile.TileContext(nc) as tc:
        _build_body(nc, tc, d)
    nc.compile()
    _CACHE["nc"] = nc
    _CACHE["dram"] = d
    return nc, d


def make_in_maps(x, gamma, beta, w0, b0, w1, b1, w2, b2, w3, b3):
    """Host-side sharding/layout prep: returns list of 8 per-core inputs."""
    e4 = ml_dtypes.float8_e4m3
    xb = np.ascontiguousarray(np.asarray(x, np.float32).reshape(B, C, N))

    cpack = np.zeros((128, CPW), np.float32)
    gamma = np.asarray(gamma, np.float32)
    beta = np.asarray(beta, np.float32)
    b0 = np.asarray(b0, np.float32)
    bout = (np.asarray(w3, np.float32) @ np.asarray(b2, np.float32)
            + np.asarray(b3, np.float32))
    for t, off in ((0, CV0), (1, CV1)):
        sl = slice(t * 128, (t + 1) * 128)
        cpack[:, off + 0] = gamma[sl]
        cpack[:, off + 1] = beta[sl]
        cpack[:, off + 2] = b0[sl]
        cpack[:, off + 3] = bout[sl]
        cpack[:, off + 4] = -gamma[sl]
    for t, off in ((0, GMA), (1, GMB)):
        ch = t * 128 + np.arange(128)
        cpack[np.arange(128), off + ch // CPG] = 1.0 / NPG
    for t, off in ((0, GTA), (1, GTB)):
        ch = t * 128 + np.arange(128)
        cpack[ch // CPG, off + np.arange(128)] = 1.0
    cpack[:, IDT:IDT + 128] = np.eye(128, dtype=np.float32)
    cpack[:, EPC] = EPS

    # wb[p, oh, kt, j] = w0^T[kt*128+p, oh*128+j]
    w0t = np.asarray(w0, np.float32).T.reshape(2, 128, 2, 128)  # [kt, p, oh, j]
    wb = w0t.transpose(1, 2, 0, 3).reshape(128, -1).astype(ml_dtypes.bfloat16)
    # w1p8[p, cs, kto, j] = w1[kto*128+p, cs*128+j]
    w1a = np.asarray(w1, np.float32).reshape(2, 128, 2, 128)    # [kto, p, cs, j]
    w1p8 = w1a.transpose(1, 2, 0, 3).reshape(128, -1).astype(e4)
    w23 = (np.asarray(w3, np.float32) @ np.asarray(w2, np.float32)).T
    w23t = w23.reshape(2, 128, 2, 128).transpose(1, 2, 0, 3)
    w23t = w23t.reshape(128, -1).astype(ml_dtypes.bfloat16)

    in_maps = []
    for core in range(NCORES):
        b, j = divmod(core, 4)
        xc = xb[b]
        xs8 = xc.reshape(2, 128, MT, 128).transpose(1, 2, 0, 3)
        xT = xc.reshape(2, 128, 16, 2, 128)  # [t, jj, pr, kt, p]
        xT = xT.transpose(4, 2, 0, 3, 1)
        xqc = xc[:, j * NSH:(j + 1) * NSH]
        xq = xqc.reshape(2, 128, NSH).transpose(1, 0, 2).reshape(128, -1)
        m = {
            "xs8": xs8.reshape(128, -1).astype(e4),
            "xTw8": xT.reshape(128, -1).astype(e4),
            "xq8": xq.astype(e4),
            "xq": np.ascontiguousarray(xq),
            "wb": wb, "w1p8": w1p8, "w23t": w23t, "cpack": cpack,
        }
        in_maps.append(m)
    return in_maps


def assemble_output(results):
    out = np.zeros((B, C, N), np.float32)
    for core in range(NCORES):
        b, j = divmod(core, 4)
        y = results[core]["y"].reshape(128, 2, NSH).transpose(1, 0, 2)
        out[b][:, j * NSH:(j + 1) * NSH] = y.reshape(C, NSH)
    return out.reshape(B, C, 16, 16, 16)


def kernel(x, gamma, beta, w0, b0, w1, b1, w2, b2, w3, b3):
    nc, _ = _get_program()
    in_maps = make_in_maps(x, gamma, beta, w0, b0, w1, b1, w2, b2, w3, b3)
    res = bass_utils.run_bass_kernel_spmd(nc, in_maps, core_ids=list(range(NCORES)))
    return assemble_output(res.results)


# revision 15
# speedup vs baseline: 1.6067x; 1.0130x over previous
"""Trainium2 Bass/Tile kernel for AttnBlock:
GroupNorm(32) -> 1x1 conv q,k,v -> softmax attention over N=4096 tokens
-> 1x1 conv proj -> residual.

Sharding: 8 cores = 2 (batch) x 4 (query-token shards of N).  Each core gets
the full x of its batch plus its n-shard slice, and produces the [C, N/4]
output shard.  No collectives.

Architecture (v4):
- All heavy matmuls are fp8 MatmulPerfMode.DoubleRow: the full K=256
  contraction in one instruction at 0.5 cycles/output-column.  DR stationary
  operands need their 256 weight elements contiguous per partition; every
  lhsT is laid out [.., kt(2), 128].
- GroupNorm stats via a PE Gram-matrix over the m-major fp8 x copy
  (diag -> sum x^2, ones-matmul -> sum x), diag extracted by one DVE
  scalar_tensor_tensor+accum per c-tile.
- No k tensor: S^T = x^T g with g = a*(w1^T q) [C, NSH] -- the PSUM->SBUF
  drain is the n-shard-sized g (2K lanes) instead of the m-sized k (8K).
  The k bias is dropped exactly (softmax shift invariance); q keeps its
  effective bias.
- No v tensor: attention accumulates over x itself:
  AVx[n, c] = sum_m E[m, n] x[c, m] (moving operand = resident xTw8),
  plus denominator columns from a tiny ones matmul per ns.  After
  normalize + transpose, ONE DoubleRow projection by w238 = a*(w3 w2)^T
  (host-folded w3@w2, scaled 2^19 for fp8) produces the output; the scale
  is undone in the final scalar_tensor_tensor against xqb = x + b3eff.
- Softmax over 2-bank [128,1024] S^T psum tiles; exp ns-subtiles split
  between ACT (true Exp -> fp8e5) and DVE (Schraudolph bits =
  round(logit*4*log2e + 60) as uint8 == fp8e5m2; e5m2 because logits span
  +-8).  Output APs are permuted so E tiles come out [ns, kt, j] -- the
  DR lhsT layout for AVx.
"""

import ml_dtypes
import numpy as np

import concourse.bacc as bacc
import concourse.mybir as mybir
import concourse.tile as tile
from concourse import bass_utils

f32 = mybir.dt.float32
bf16 = mybir.dt.bfloat16
fp8 = mybir.dt.float8e4
fp8e5 = mybir.dt.float8e5
u8 = mybir.dt.uint8
AF = mybir.ActivationFunctionType
ALU = mybir.AluOpType
DR = mybir.MatmulPerfMode.DoubleRow

B = 2
C = 256
N = 4096          # 16**3 tokens
NSH = N // 4      # 1024 tokens per core
G = 32
CPG = C // G      # channels per group
NPG = CPG * N     # elements per group
EPS = 1e-6
SCALE = C ** -0.5          # 1/16
LOG2E = float(1.0 / np.log(2.0))
WS = 524288.0              # 2^19 fp8-range scale on w23; undone in the stt
MT = N // 128              # 32 m-tiles

NCORES = 8

# cpack column layout
CV0, CV1 = 0, 8            # cvec slice0/1: [gamma, beta, b0, bout, -gamma]
GMA, GMB = 16, 48          # gmask per slice [128, 32] (1/NPG folded)
GTA, GTB = 80, 208         # gmaskT per slice [32, 128] on partitions 0:32
MZL = 336                  # zeros[128] | ident[128] | zeros[128]
IDT = 464
EPC = 720                  # eps column
CPW = 728

# engine splits (True -> ACT, False -> DVE)
EXP_SPLIT = [2] * 32       # of 4 ns-subtiles per (half*16+pair), how many ACT
GEP_ACT = [True, False]    # g drain per c-slice
ON_ACT = [True] * 8        # normalize per (half*4 + ns)
TRP_ACT = [True, False, True, False]  # attnx drain per (half*2 + t)

N_WARMUP = 42
PHASE = 4


def _build_body(nc, tc, d):
    from contextlib import ExitStack

    ctx = ExitStack()
    pc = ctx.enter_context(tc.tile_pool(name="const", bufs=1))
    pb = ctx.enter_context(tc.tile_pool(name="big", bufs=1))
    pw = ctx.enter_context(tc.tile_pool(name="work", bufs=3))
    ptiny = ctx.enter_context(tc.tile_pool(name="tiny", bufs=2))
    # PSUM: sp = 2 x [128,1024] (2 banks each), ot = 4 x [128,512] (1 bank)
    sp = ctx.enter_context(tc.tile_pool(name="sp", bufs=2, space="PSUM"))
    ot = ctx.enter_context(tc.tile_pool(name="pot", bufs=4, space="PSUM"))

    # ---- tiny consts ----
    zcol = pc.tile([128, 1], f32, tag="zcol", name="zcol")
    nc.vector.memset(zcol[:], 0.0)
    nc.const_aps.aps[(f32, 0.0)] = zcol[:]
    ones4 = pc.tile([128, 2, 1], fp8, tag="ones4", name="ones4")
    nc.vector.memset(ones4[:], 1.0)
    ones5 = pc.tile([128, 2, 1], fp8e5, tag="ones5", name="ones5")
    nc.vector.memset(ones5[:], 1.0)

    # ---- PE warmup: dep-free matmuls bridge the DMA head + pstate ramp
    wdum = pc.tile([128, 128], bf16, tag="wdum", name="wdum")
    nc.vector.memset(wdum[:], 1.0)
    wslot = ot.tile([128, 512], f32, tag="warm", name="warm")
    for i in range(N_WARMUP):
        nc.tensor.matmul(wslot[:, 0:128], wdum[:], wdum[:],
                         start=True, stop=True)

    # ---- input DMAs: the DMA fabric is serial -- order by need.
    # xTw8[p, pr, t, kt, j] = x[t*128+j, (2*pr+kt)*128+p], in quarters
    xTw8 = pb.tile([128, 16, 2, 2, 128], fp8, tag="xTw8", name="xTw8")
    xTw8f = xTw8[:].rearrange("p a b c e -> p (a b c e)")
    qs = [nc.sync, nc.scalar]
    cpack = pc.tile([128, CPW], f32, tag="cpack", name="cpack")
    for qr in range(4):
        qs[qr % 2].dma_start(xTw8f[:, qr * 2048:(qr + 1) * 2048],
                             d["xTw8"][:, qr * 2048:(qr + 1) * 2048])
        if qr == 1:
            nc.sync.dma_start(cpack[:], d["cpack"][:])
        if qr == 2:
            xq8 = pb.tile([128, 2, NSH], fp8, tag="xq8", name="xq8")
            nc.scalar.dma_start(xq8[:].rearrange("p a b -> p (a b)"),
                                d["xq8"][:])
    # wb[p, oh, kt, j] = w0^T[kt*128+p, oh*128+j]
    wb = pb.tile([128, 2, 2, 128], bf16, tag="wb", name="wb")
    nc.sync.dma_start(wb[:].rearrange("p a b c -> p (a b c)"), d["wb"][:])
    # w1p8[p, cs, kto, j] = w1[kto*128+p, cs*128+j]  (plain w1, fp8)
    w1p8 = pb.tile([128, 2, 2, 128], fp8, tag="w1p8", name="w1p8")
    nc.scalar.dma_start(w1p8[:].rearrange("p a b c -> p (a b c)"), d["w1p8"][:])
    # w23t[p, oh, kt, j] = (w3 w2)^T[kt*128+p, oh*128+j]
    w23t = pb.tile([128, 2, 2, 128], bf16, tag="w23t", name="w23t")
    nc.sync.dma_start(w23t[:].rearrange("p a b c -> p (a b c)"), d["w23t"][:])
    # xs8[p, mt, kt, j] = x[kt*128+p, mt*128+j], halves
    xs8 = pb.tile([128, MT, 2, 128], fp8, tag="xs8", name="xs8")
    xs8f = xs8[:].rearrange("p a b c -> p (a b c)")
    nc.scalar.dma_start(xs8f[:, 0:4096], d["xs8"][:, 0:4096])
    nc.sync.dma_start(xs8f[:, 4096:8192], d["xs8"][:, 4096:8192])
    # xq (f32 residual) is emitted LAST -- only needed by the final stt

    cvec = [cpack[:, CV0:CV0 + 8], cpack[:, CV1:CV1 + 8]]
    gm = [cpack[:, GMA:GMA + 32], cpack[:, GMB:GMB + 32]]
    gmt = [cpack[0:32, GTA:GTA + 128], cpack[0:32, GTB:GTB + 128]]
    ident = cpack[:, IDT:IDT + 128]
    dmask = [cpack[:, IDT:IDT + 256], cpack[:, MZL:MZL + 256]]
    epscol = cpack[0:32, EPC:EPC + 1]

    # ---- GroupNorm stats via PE Gram over xTw8 ----
    # gtile: [t0 gram 0:256 | t0 sum-x 256 | pad | t1 gram 512:768 | t1 sum-x]
    gtile = sp.tile([128, 1024], f32, tag="sp", name="gram")
    goff = [0, 512]
    for pr in range(16):
        for t in range(2):
            lhs = xTw8[:, pr, t]
            nc.tensor.matmul(
                gtile[:, goff[t]:goff[t] + 256],
                lhs,
                xTw8[:, pr].rearrange("p t kt j -> p kt t j"),
                start=(pr == 0), stop=(pr == 15), perf_mode=DR,
            )
            nc.tensor.matmul(
                gtile[:, goff[t] + 256:goff[t] + 257],
                lhs, ones4[:],
                start=(pr == 0), stop=(pr == 15), perf_mode=DR,
            )
    # pt[t]: col0 = sum x, col1 = sum x^2 (diag extract)
    pt = [ptiny.tile([128, 2], f32, tag=f"pt{t}", name=f"pt{t}") for t in range(2)]
    trash = pw.tile([128, 256], f32, tag="trash", name="trash", bufs=2)
    for t in range(2):
        nc.vector.tensor_copy(pt[t][:, 0:1], gtile[:, goff[t] + 256:goff[t] + 257])
        nc.vector.scalar_tensor_tensor(
            trash[:], gtile[:, goff[t]:goff[t] + 256], 1.0, dmask[t],
            op0=ALU.mult, op1=ALU.mult, accum_out=pt[t][:, 1:2],
        )

    # group stats -> mu/rstd -> per-channel a, bfold
    s32 = ot.tile([128, 512], f32, tag="warm", name="s32")
    for t in range(2):
        nc.tensor.matmul(s32[0:32, 0:2], gm[t], pt[t][:],
                         start=(t == 0), stop=(t == 1))
    sg = ptiny.tile([32, 2], f32, tag="sg", name="sg")
    nc.vector.tensor_copy(sg[:], s32[0:32, 0:2])
    mr = ptiny.tile([32, 2], f32, tag="mr", name="mr")
    musq = ptiny.tile([32, 1], f32, tag="musq", name="musq")
    nc.vector.tensor_mul(musq[:], sg[:, 0:1], sg[:, 0:1])
    var = ptiny.tile([32, 1], f32, tag="var", name="var")
    nc.vector.tensor_sub(var[:], sg[:, 1:2], musq[:])
    std = ptiny.tile([32, 1], f32, tag="std", name="std")
    nc.scalar.activation(std[:], var[:], AF.Sqrt, bias=epscol)
    # dummy Exp preloads the exp table during the DMA head
    edum = ptiny.tile([128, 1], u8, tag="edum", name="edum")
    nc.scalar.activation(edum[:].bitcast(fp8e5), zcol[:], AF.Exp)
    nc.vector.reciprocal(mr[:, 1:2], std[:])
    nc.vector.tensor_mul(mr[:, 0:1], sg[:, 0:1], mr[:, 1:2])

    a_t, bfold_bf = [], []
    bc = ot.tile([128, 512], f32, tag="warm", name="bc")
    for t in range(2):
        nc.tensor.matmul(bc[:, 2 * t:2 * t + 2], gmt[t], mr[:],
                         start=True, stop=True)
    for t in range(2):
        a = pb.tile([128, 1], f32, tag=f"a{t}", name=f"a{t}")
        nc.vector.tensor_mul(a[:], bc[:, 2 * t + 1:2 * t + 2], cvec[t][:, 0:1])
        bf = pb.tile([128, 1], f32, tag=f"bf{t}", name=f"bf{t}")
        nc.vector.tensor_scalar(
            bf[:], bc[:, 2 * t:2 * t + 1], cvec[t][:, 4:5], cvec[t][:, 1:2],
            op0=ALU.mult, op1=ALU.add,
        )
        bb = pb.tile([128, 1], bf16, tag=f"bfb{t}", name=f"bfb{t}")
        nc.vector.tensor_copy(bb[:], bf[:])
        a_t.append(a)
        bfold_bf.append(bb)

    def _early_out(srcs):
        for oh in range(2):
            for ch in range(2):
                yt = pw.tile([128, 512], f32, tag="yt", name="yt")
                nc.vector.tensor_copy(
                    yt[:], srcs[oh][:, ch * 512:(ch + 1) * 512])
                nc.sync.dma_start(
                    d["y"][:, oh * NSH + ch * 512:oh * NSH + (ch + 1) * 512],
                    yt[:])

    # ---- effective biases (RAW weights -- emitted before the a-fold) ----
    beff0 = []
    for oh in range(2):
        bp = ot.tile([128, 512], f32, tag="warm", name="bp")
        for t in range(2):
            nc.tensor.matmul(bp[:, 0:1], wb[:, oh, t],
                             bfold_bf[t][:], start=(t == 0), stop=(t == 1))
        bs = pb.tile([128, 1], f32, tag=f"beff0_{oh}", name=f"beff0_{oh}")
        nc.scalar.activation(bs[:], bp[:, 0:1], AF.Identity,
                             bias=cvec[oh][:, 2:3])
        beff0.append(bs)
    b3eff = []
    for oh in range(2):
        bp = ot.tile([128, 512], f32, tag="warm", name="bp3")
        for t in range(2):
            nc.tensor.matmul(bp[:, 0:1], w23t[:, oh, t],
                             bfold_bf[t][:], start=(t == 0), stop=(t == 1))
        bs = pb.tile([128, 1], f32, tag=f"b3eff{oh}", name=f"b3eff{oh}")
        nc.scalar.activation(bs[:], bp[:, 0:1], AF.Identity,
                             bias=cvec[oh][:, 3:4])
        b3eff.append(bs)

    # xqb = x-shard + b3eff (f32); the final stt adds proj*2^-19 onto it.
    # xq is the last input DMA issued -- its transfer queues behind all the
    # early-needed inputs on the serial DMA fabric but lands well before the
    # first finish phase needs xqb.
    xq = pb.tile([128, 2, NSH], f32, tag="xq", name="xq")
    nc.sync.dma_start(xq[:].rearrange("p a b -> p (a b)"), d["xq"][:])
    f32r = mybir.dt.float32r
    xqb = pb.tile([128, 2, NSH], f32r, tag="xqb", name="xqb")
    for t in range(2):
        nc.vector.tensor_scalar(xqb[:, t], xq[:, t], b3eff[t][:], WS,
                                op0=ALU.add, op1=ALU.mult)
    identr = pb.tile([128, 128], f32r, tag="identr", name="identr")
    nc.vector.tensor_copy(identr[:], ident)

    # ---- fold a into w0 (in place, bf16) -> fp8; w238 = a*(w3 w2)^T * WS ----
    w018 = pb.tile([128, 2, 2, 128], fp8, tag="w018", name="w018")
    w23s = pb.tile([128, 2, 2, 128], bf16, tag="w23s", name="w23s")
    w238 = pb.tile([128, 2, 2, 128], fp8, tag="w238", name="w238")
    for t in range(2):
        nc.vector.tensor_scalar_mul(wb[:, :, t], wb[:, :, t], a_t[t][:])
        nc.gpsimd.tensor_copy(w018[:, :, t], wb[:, :, t])
        nc.vector.tensor_scalar(w23s[:, :, t], w23t[:, :, t], a_t[t][:], WS,
                                op0=ALU.mult, op1=ALU.mult)
        nc.gpsimd.tensor_copy(w238[:, :, t], w23s[:, :, t])

    if PHASE <= 1:
        _early_out([xq[:, 0], xq[:, 1]])
        ctx.close()
        return

    # ---- q = w0a @ xq + beff0 : fp8 [128, 2(kt=oh), NSH] ----
    q2 = pb.tile([128, 2, NSH], fp8, tag="q2", name="q2")
    for oh in range(2):
        qp = sp.tile([128, 1024], f32, tag="sp", name="qp")
        for ch in range(2):
            nc.tensor.matmul(
                qp[:, ch * 512:(ch + 1) * 512],
                w018[:, oh],
                xq8[:, :, ch * 512:(ch + 1) * 512],
                start=True, stop=True, perf_mode=DR,
            )
        if oh == 0:
            nc.scalar.activation(q2[:, oh, :], qp[:], AF.Identity,
                                 bias=beff0[oh][:])
        else:
            nc.vector.tensor_scalar(q2[:, oh, :], qp[:], beff0[oh][:], None,
                                    op0=ALU.add)

    # ---- g = a * (w1^T q) : fp8 [128, 2(kt=c-slice), NSH] ----
    g8 = pb.tile([128, 2, NSH], fp8, tag="g8", name="g8")
    for cs in range(2):
        gp = sp.tile([128, 1024], f32, tag="sp", name="gp")
        for h in range(2):
            nc.tensor.matmul(
                gp[:, h * 512:(h + 1) * 512],
                w1p8[:, cs],
                q2[:, :, h * 512:(h + 1) * 512],
                start=True, stop=True, perf_mode=DR,
            )
        if GEP_ACT[cs]:
            nc.scalar.activation(g8[:, cs, :], gp[:], AF.Copy,
                                 scale=a_t[cs][:])
        else:
            nc.vector.tensor_scalar_mul(g8[:, cs, :], gp[:], a_t[cs][:])

    if PHASE == 2:
        _early_out([xq[:, 0], xq[:, 1]])
        ctx.close()
        return

    # ---- attention ----
    yts = [pb.tile([128, NSH], f32, tag=f"yts{t}", name=f"yts{t}")
           for t in range(2)]
    # attnx8[p, t(kt for proj), half, n] -- normalized attention-averaged x
    attnx8 = pb.tile([128, 2, 2, 512], fp8, tag="attnx8", name="attnx8")

    def emit_s_exp(half, pr):
        # S^T pair: S[m, n] = sum_c x[c, m] g[c, n], then exp -> E chunk
        st = sp.tile([128, 1024], f32, tag="sp", name="st")
        for h in range(2):
            nc.tensor.matmul(
                st[:, h * 512:(h + 1) * 512],
                xs8[:, 2 * pr + h],
                g8[:, :, half * 512:(half + 1) * 512],
                start=True, stop=True, perf_mode=DR,
            )
        # exp -> fp8e5 E chunk, permuted out to [p, ns, kt(2 mt), j]
        e = pw.tile([128, 4, 2, 128], u8, tag="e", name="e", bufs=5)
        stv = st[:].rearrange("p (kt ns j) -> p kt ns j", kt=2, ns=4)
        ev = e[:].rearrange("p ns kt j -> p kt ns j")
        a = EXP_SPLIT[half * 16 + pr]
        if a > 0:
            nc.scalar.activation(ev.bitcast(fp8e5)[:, :, 0:a, :],
                                 stv[:, :, 0:a, :], AF.Exp, scale=SCALE)
        if a < 4:
            nc.vector.tensor_scalar(ev[:, :, a:4, :], stv[:, :, a:4, :],
                                    SCALE * 4.0 * LOG2E, 60.0,
                                    op0=ALU.mult, op1=ALU.add)
        return e

    def emit_avx(half, pr, e, ots):
        # AVx accumulation + denominator columns (in ots[0] cols 256:260)
        xr = xTw8[:, pr].rearrange("p t kt j -> p kt t j")
        for ns in range(4):
            el = e[:, ns].bitcast(fp8e5)
            nc.tensor.matmul(
                ots[ns][:, 0:256], el, xr,
                start=(pr == 0), stop=(pr == 15), perf_mode=DR,
            )
            nc.tensor.matmul(
                ots[0][:, 256 + ns:257 + ns], el, ones5[:],
                start=(pr == 0), stop=(pr == 15), perf_mode=DR,
            )


    def finish_steps(half, ots):
        # generator of emission steps; each step is interleaved between the
        # next half's pairs so no engine queue sees a serial finish block.
        rec4 = ptiny.tile([128, 4], f32, tag="rec4", name="rec4")
        nc.vector.reciprocal(rec4[:], ots[0][:, 256:260])
        trps = [None, None]
        ons = []

        def emit_on(ns):
            on = pw.tile([128, 256], f32, tag="on", name="on", bufs=5)
            if ON_ACT[half * 4 + ns]:
                nc.scalar.activation(on[:], ots[ns][:, 0:256], AF.Identity,
                                     scale=rec4[:, ns:ns + 1])
            else:
                nc.vector.tensor_scalar_mul(on[:], ots[ns][:, 0:256],
                                            rec4[:, ns:ns + 1])
            ons.append(on)
            if ns == 0:
                trps[0] = ot.tile([128, 512], f32, tag="warm",
                                  name=f"trp{half}_0")
                trps[1] = ot.tile([128, 512], f32, tag="warm",
                                  name=f"trp{half}_1")
            for t in range(2):
                nc.tensor.transpose(trps[t][:, ns * 128:(ns + 1) * 128],
                                    ons[ns][:, t * 128:(t + 1) * 128], ident)

        def emit_drain(t):
            dst = attnx8[:, t, half]
            if TRP_ACT[half * 2 + t]:
                nc.scalar.activation(dst, trps[t][:], AF.Copy)
            else:
                nc.vector.tensor_copy(dst, trps[t][:])

        def emit_proj(oh):
            # residual preload (exact f32 via f32r identity matmul), then the
            # DR projection accumulates the attention output on top.
            pj = ot.tile([128, 512], f32, tag="warm", name=f"pj{half}_{oh}")
            nc.tensor.matmul(pj[:], identr[:],
                             xqb[:, oh, half * 512:(half + 1) * 512],
                             start=True, stop=False, skip_group_check=True)
            nc.tensor.matmul(pj[:], w238[:, oh], attnx8[:, :, half],
                             start=False, stop=True, perf_mode=DR,
                             skip_group_check=True)
            sl = yts[oh][:, half * 512:(half + 1) * 512]
            if (half + oh) % 2 == 0:
                nc.scalar.activation(sl, pj[:], AF.Copy, scale=1.0 / WS)
            else:
                nc.vector.tensor_scalar_mul(sl, pj[:], 1.0 / WS)
            dq = (nc.sync, nc.scalar)[(half + oh) % 2]
            dq.dma_start(
                d["y"][:, oh * NSH + half * 512:oh * NSH + (half + 1) * 512],
                sl,
            )

        yield lambda: emit_on(0)
        yield lambda: emit_on(1)
        yield lambda: emit_on(2)
        yield lambda: emit_on(3)
        yield lambda: emit_drain(0)
        yield lambda: emit_drain(1)
        yield lambda: emit_proj(0)
        yield lambda: emit_proj(1)

    pending = None
    for half in range(2):
        ots = [ot.tile([128, 512], f32, tag="warm", name=f"ot{half}_{ns}")
               for ns in range(4)]
        e_cur = emit_s_exp(half, 0)
        for pr in range(16):
            e_nxt = emit_s_exp(half, pr + 1) if pr + 1 < 16 else None
            emit_avx(half, pr, e_cur, ots)
            e_cur = e_nxt
            if pending is not None:
                nxt = next(pending, None)
                if nxt is None:
                    pending = None
                else:
                    nxt()
        pending = finish_steps(half, ots)
    for step in pending:
        step()

    ctx.close()


_CACHE = {}


def _get_program():
    if "nc" in _CACHE:
        return _CACHE["nc"], _CACHE["dram"]
    nc = bacc.Bacc("TRN2", target_bir_lowering=False, debug=False,
                   enable_asserts=False, num_devices=NCORES)
    d = {}
    d["xs8"] = nc.dram_tensor("xs8", [128, MT * 256], fp8,
                              kind="ExternalInput").ap()
    d["xTw8"] = nc.dram_tensor("xTw8", [128, 16 * 512], fp8,
                               kind="ExternalInput").ap()
    d["xq8"] = nc.dram_tensor("xq8", [128, 2 * NSH], fp8,
                              kind="ExternalInput").ap()
    d["xq"] = nc.dram_tensor("xq", [128, 2 * NSH], f32, kind="ExternalInput").ap()
    d["wb"] = nc.dram_tensor("wb", [128, 4 * 128], bf16, kind="ExternalInput").ap()
    d["w1p8"] = nc.dram_tensor("w1p8", [128, 4 * 128], fp8,
                               kind="ExternalInput").ap()
    d["w23t"] = nc.dram_tensor("w23t", [128, 4 * 128], bf16,
                               kind="ExternalInput").ap()
    d["cpack"] = nc.dram_tensor("cpack", [128, CPW], f32,
                                kind="ExternalInput").ap()
    d["y"] = nc.dram_tensor("y", [128, 2 * NSH], f32, kind="ExternalOutput").ap()

    with tile.TileContext(nc) as tc:
        _build_body(nc, tc, d)
    nc.compile()
    _CACHE["nc"] = nc
    _CACHE["dram"] = d
    return nc, d


def make_in_maps(x, gamma, beta, w0, b0, w1, b1, w2, b2, w3, b3):
    """Host-side sharding/layout prep: returns list of 8 per-core inputs."""
    e4 = ml_dtypes.float8_e4m3
    xb = np.ascontiguousarray(np.asarray(x, np.float32).reshape(B, C, N))

    cpack = np.zeros((128, CPW), np.float32)
    gamma = np.asarray(gamma, np.float32)
    beta = np.asarray(beta, np.float32)
    b0 = np.asarray(b0, np.float32)
    bout = (np.asarray(w3, np.float32) @ np.asarray(b2, np.float32)
            + np.asarray(b3, np.float32))
    for t, off in ((0, CV0), (1, CV1)):
        sl = slice(t * 128, (t + 1) * 128)
        cpack[:, off + 0] = gamma[sl]
        cpack[:, off + 1] = beta[sl]
        cpack[:, off + 2] = b0[sl]
        cpack[:, off + 3] = bout[sl]
        cpack[:, off + 4] = -gamma[sl]
    for t, off in ((0, GMA), (1, GMB)):
        ch = t * 128 + np.arange(128)
        cpack[np.arange(128), off + ch // CPG] = 1.0 / NPG
    for t, off in ((0, GTA), (1, GTB)):
        ch = t * 128 + np.arange(128)
        cpack[ch // CPG, off + np.arange(128)] = 1.0
    cpack[:, IDT:IDT + 128] = np.eye(128, dtype=np.float32)
    cpack[:, EPC] = EPS

    # wb[p, oh, kt, j] = w0^T[kt*128+p, oh*128+j]
    w0t = np.asarray(w0, np.float32).T.reshape(2, 128, 2, 128)  # [kt, p, oh, j]
    wb = w0t.transpose(1, 2, 0, 3).reshape(128, -1).astype(ml_dtypes.bfloat16)
    # w1p8[p, cs, kto, j] = w1[kto*128+p, cs*128+j]
    w1a = np.asarray(w1, np.float32).reshape(2, 128, 2, 128)    # [kto, p, cs, j]
    w1p8 = w1a.transpose(1, 2, 0, 3).reshape(128, -1).astype(e4)
    w23 = (np.asarray(w3, np.float32) @ np.asarray(w2, np.float32)).T
    w23t = w23.reshape(2, 128, 2, 128).transpose(1, 2, 0, 3)
    w23t = w23t.reshape(128, -1).astype(ml_dtypes.bfloat16)

    in_maps = []
    for core in range(NCORES):
        b, j = divmod(core, 4)
        xc = xb[b]
        xs8 = xc.reshape(2, 128, MT, 128).transpose(1, 2, 0, 3)
        xT = xc.reshape(2, 128, 16, 2, 128)  # [t, jj, pr, kt, p]
        xT = xT.transpose(4, 2, 0, 3, 1)
        xqc = xc[:, j * NSH:(j + 1) * NSH]
        xq = xqc.reshape(2, 128, NSH).transpose(1, 0, 2).reshape(128, -1)
        m = {
            "xs8": xs8.reshape(128, -1).astype(e4),
            "xTw8": xT.reshape(128, -1).astype(e4),
            "xq8": xq.astype(e4),
            "xq": np.ascontiguousarray(xq),
            "wb": wb, "w1p8": w1p8, "w23t": w23t, "cpack": cpack,
        }
        in_maps.append(m)
    return in_maps


def assemble_output(results):
    out = np.zeros((B, C, N), np.float32)
    for core in range(NCORES):
        b, j = divmod(core, 4)
        y = results[core]["y"].reshape(128, 2, NSH).transpose(1, 0, 2)
        out[b][:, j * NSH:(j + 1) * NSH] = y.reshape(C, NSH)
    return out.reshape(B, C, 16, 16, 16)


def kernel(x, gamma, beta, w0, b0, w1, b1, w2, b2, w3, b3):
    nc, _ = _get_program()
    in_maps = make_in_maps(x, gamma, beta, w0, b0, w1, b1, w2, b2, w3, b3)
    res = bass_utils.run_bass_kernel_spmd(nc, in_maps, core_ids=list(range(NCORES)))
    return assemble_output(res.results)
